# revision 1
# baseline (speedup 1.0000x reference)
"""2-layer GCN (GCNConv+relu x2, linear head) on 8 Trainium2 NeuronCores.

Strategy (graph/data parallel, per sharding hint):
  - Nodes sharded across 8 cores by id; edges partitioned by destination.
  - Per core, destination nodes are bin-packed into B_FIX blocks of <=BLK
    dsts such that each (block, source-window) holds <= KCOL*128 edges.
    This gives an SPMD-uniform program; only tensor data varies per core.
  - Per layer: local matmul (x@W scaled by dinv) -> AllGather into a
    full node-major table in DRAM -> dma_gather messages per edge slot
    (4 source windows to satisfy the int16 index range) -> selection
    matrix S built with one batched is_equal DVE op per gather batch ->
    PE matmuls (lhsT=messages, rhs=S) accumulate feature-major conv
    output in PSUM; self-loop terms enter via an identity-matmul
    transpose. Post: scale by dinv, +bias, relu, next-layer matmul.
"""

import numpy as np

import concourse.bass as bass
import concourse.mybir as mybir
import concourse.tile as tile
from concourse import bacc
from concourse import bass_utils

import ml_dtypes

F32 = mybir.dt.float32
BF16 = mybir.dt.bfloat16
I16 = mybir.dt.int16
NP_BF16 = ml_dtypes.bfloat16


class Cfg:
    def __init__(self, n_nodes, in_feat, hidden, n_classes, n_cores, n_c,
                 blk, kcol, b_fix, nq, c_batch, self_dtype="bf16"):
        self.N = n_nodes
        self.IN_FEAT = in_feat
        self.HIDDEN = hidden
        self.N_CLASSES = n_classes
        self.NC = n_cores
        self.N_C = n_c                    # nodes per core (id // N_C)
        assert n_c * n_cores >= n_nodes
        self.BLK = blk                    # max dsts per block
        self.KCOL = kcol                  # columns per (block, stream)
        self.CAP = kcol * 128             # max edges per (block, stream)
        self.B_FIX = b_fix                # blocks per core (uniform)
        self.NQ = nq                      # source windows / gather streams
        self.SLOTS_C = b_fix * blk        # table slots per core
        assert self.SLOTS_C % 128 == 0
        self.NT = self.SLOTS_C // 128     # node tiles per core
        assert self.NT % 2 == 0
        self.TABLE_N = n_cores * self.SLOTS_C
        assert self.TABLE_N % nq == 0
        self.WIN = self.TABLE_N // nq     # table rows per source window
        assert self.WIN <= 32767          # int16 gather index range
        assert (n_cores % nq) == 0
        self.COLS_Q = b_fix * kcol        # gather columns per stream
        self.C_BATCH = c_batch            # columns per gather batch
        assert c_batch % kcol == 0 and self.COLS_Q % c_batch == 0
        self.N_BATCH = self.COLS_Q // c_batch
        self.BPB = c_batch // kcol        # blocks per batch
        assert self.BPB % 2 == 0          # block pairs never straddle batches
        self.SELF_DT = BF16 if self_dtype == "bf16" else F32
        self.NP_SELF = NP_BF16 if self_dtype == "bf16" else np.float32


CFG_FULL = Cfg(n_nodes=100000, in_feat=128, hidden=64, n_classes=16,
               n_cores=8, n_c=12544, blk=64, kcol=2, b_fix=224, nq=4,
               c_batch=28)


# ---------------------------------------------------------------------------
# Host-side preprocessing (sharding): all integer graph restructuring.
# ---------------------------------------------------------------------------

def preprocess(cfg, x, edge_index, W1, b1, W2, b2, Wl, bl):
    N, NC, N_C = cfg.N, cfg.NC, cfg.N_C
    src = np.asarray(edge_index[0]).astype(np.int64)
    dst = np.asarray(edge_index[1]).astype(np.int64)
    x = np.asarray(x, dtype=np.float32)

    deg = np.bincount(dst, minlength=N).astype(np.float32) + 1.0
    dinv = (1.0 / np.sqrt(deg)).astype(np.float32)

    cores_per_q = NC // cfg.NQ
    q_of = (src // N_C) // cores_per_q       # stream of each edge

    # per-(node, q) incoming edge counts
    degq = np.bincount(dst * cfg.NQ + q_of, minlength=N * cfg.NQ)\
             .reshape(N, cfg.NQ)

    # --- per-core first-fit-decreasing packing of dsts into blocks ---
    slot_of = np.full(NC * N_C, -1, dtype=np.int64)
    node_of_slot = np.full(cfg.TABLE_N, -1, dtype=np.int64)
    for c in range(NC):
        lo, hi = c * N_C, min((c + 1) * N_C, N)
        n_here = hi - lo
        if n_here <= 0:
            continue
        dq = degq[lo:hi]
        order = np.argsort(-dq.max(axis=1), kind="stable")
        accs = np.zeros((cfg.B_FIX, cfg.NQ), dtype=np.int64)
        cnts = np.zeros(cfg.B_FIX, dtype=np.int64)
        nopen = 1
        for j in order:
            v = dq[j]
            fits = (cnts[:nopen] < cfg.BLK) & \
                   np.all(accs[:nopen] + v <= cfg.CAP, axis=1)
            w = np.flatnonzero(fits)
            if w.size == 0:
                assert nopen < cfg.B_FIX, \
                    f"core {c}: packing exceeds {cfg.B_FIX} blocks"
                b = nopen
                nopen += 1
            else:
                b = int(w[0])
            g = lo + j
            s = c * cfg.SLOTS_C + b * cfg.BLK + cnts[b]
            slot_of[g] = s
            node_of_slot[s] = g
            accs[b] += v
            cnts[b] += 1

    slot_of = slot_of[:N]

    # --- per-core edge streams ---
    e_core = dst // N_C
    s_slot = slot_of[src]
    d_slot_l = slot_of[dst] - e_core * cfg.SLOTS_C
    e_b = d_slot_l // cfg.BLK
    e_r = d_slot_l % cfg.BLK

    P_Q = cfg.B_FIX * cfg.CAP            # positions per stream
    idx_all = np.zeros((NC, cfg.NQ, P_Q), dtype=np.int16)
    dl_all = np.full((NC, cfg.NQ, P_Q), 255.0, dtype=np.float32)

    order2 = np.lexsort((e_b, q_of, e_core))
    es_c, eq_c, eb_c = e_core[order2], q_of[order2], e_b[order2]
    grp = (es_c * cfg.NQ + eq_c) * cfg.B_FIX + eb_c
    _, start_idx, cnt_grp = np.unique(grp, return_index=True,
                                      return_counts=True)
    rank = np.arange(grp.size) - np.repeat(start_idx, cnt_grp)
    assert rank.max(initial=0) < cfg.CAP
    pos = eb_c * cfg.CAP + rank
    idx_val = (s_slot[order2] % cfg.WIN).astype(np.int16)
    idx_all[es_c, eq_c, pos] = idx_val
    dl_all[es_c, eq_c, pos] = e_r[order2].astype(np.float32)

    # wrapped int16 layout: position i -> [i%16, i//16], replicated x8
    idx_w = idx_all.reshape(NC, cfg.NQ, -1, 16).transpose(0, 1, 3, 2)
    idx_dev = np.ascontiguousarray(np.tile(idx_w, (1, 1, 8, 1)))
    # dstloc layout: position -> [pos%128, pos//128]
    dl_dev = np.ascontiguousarray(
        dl_all.reshape(NC, cfg.NQ, cfg.COLS_Q, 128).transpose(0, 1, 3, 2))

    # --- per-slot node data ---
    valid = node_of_slot >= 0
    xe = np.zeros((cfg.TABLE_N, cfg.IN_FEAT), dtype=np.float32)
    xe[valid] = x[node_of_slot[valid]]
    dinv_s = np.zeros(cfg.TABLE_N, dtype=np.float32)
    dinv_s[valid] = dinv[node_of_slot[valid]]

    W1 = np.asarray(W1, np.float32)
    W2 = np.asarray(W2, np.float32)
    Wl = np.asarray(Wl, np.float32)
    b1 = np.asarray(b1, np.float32)
    b2 = np.asarray(b2, np.float32)
    bl = np.asarray(bl, np.float32)

    iota64 = np.tile(np.arange(cfg.BLK, dtype=np.float32)[None, :], (128, 1))
    ident2 = np.concatenate([np.eye(cfg.HIDDEN), np.eye(cfg.HIDDEN)],
                            axis=0).astype(cfg.NP_SELF)

    in_maps = []
    for c in range(NC):
        sl = slice(c * cfg.SLOTS_C, (c + 1) * cfg.SLOTS_C)
        dv = dinv_s[sl]
        m = {
            "xT": np.ascontiguousarray(xe[sl].T),
            "w1": W1, "w2": W2, "wl": Wl,
            "b1c": b1.reshape(-1, 1), "b2c": b2.reshape(-1, 1),
            "blrep": np.tile(bl[None, :], (128, 1)),
            "dinvn": np.ascontiguousarray(dv.reshape(cfg.NT, 128).T),
            "dinv2n": np.ascontiguousarray((dv * dv).reshape(cfg.NT, 128).T),
            "dinvfm": np.tile(dv[None, :], (cfg.HIDDEN, 1)),
            "iota64": iota64,
            "ident2": ident2,
        }
        for q in range(cfg.NQ):
            m[f"idx{q}"] = idx_dev[c, q]
            m[f"dl{q}"] = dl_dev[c, q]
        in_maps.append(m)

    return in_maps, node_of_slot


def assemble_output(cfg, results, node_of_slot):
    out = np.zeros((cfg.N, cfg.N_CLASSES), dtype=np.float32)
    for c, r in enumerate(results):
        lg = r["logits"].reshape(128, cfg.NT, cfg.N_CLASSES)
        sl = node_of_slot[c * cfg.SLOTS_C:(c + 1) * cfg.SLOTS_C]\
            .reshape(cfg.NT, 128)
        for t in range(cfg.NT):
            v = sl[t] >= 0
            out[sl[t][v]] = lg[v, t, :]
    return out


# ---------------------------------------------------------------------------
# Device program
# ---------------------------------------------------------------------------

def build_program(cfg):
    nc = bacc.Bacc("TRN2", target_bir_lowering=False, debug=False,
                   num_devices=cfg.NC, num_swdge_queues=1)
    H, NT = cfg.HIDDEN, cfg.NT

    xT_d = nc.dram_tensor("xT", [cfg.IN_FEAT, cfg.SLOTS_C], F32,
                          kind="ExternalInput")
    w1_d = nc.dram_tensor("w1", [cfg.IN_FEAT, H], F32, kind="ExternalInput")
    w2_d = nc.dram_tensor("w2", [H, H], F32, kind="ExternalInput")
    wl_d = nc.dram_tensor("wl", [H, cfg.N_CLASSES], F32, kind="ExternalInput")
    b1c_d = nc.dram_tensor("b1c", [H, 1], F32, kind="ExternalInput")
    b2c_d = nc.dram_tensor("b2c", [H, 1], F32, kind="ExternalInput")
    blrep_d = nc.dram_tensor("blrep", [128, cfg.N_CLASSES], F32,
                             kind="ExternalInput")
    dinvn_d = nc.dram_tensor("dinvn", [128, NT], F32, kind="ExternalInput")
    dinv2n_d = nc.dram_tensor("dinv2n", [128, NT], F32, kind="ExternalInput")
    dinvfm_d = nc.dram_tensor("dinvfm", [H, cfg.SLOTS_C], F32,
                              kind="ExternalInput")
    iota_d = nc.dram_tensor("iota64", [128, cfg.BLK], F32,
                            kind="ExternalInput")
    ident_d = nc.dram_tensor("ident2", [128, H], cfg.SELF_DT,
                             kind="ExternalInput")
    idx_d = [nc.dram_tensor(f"idx{q}", [128, cfg.COLS_Q * 8], I16,
                            kind="ExternalInput") for q in range(cfg.NQ)]
    dl_d = [nc.dram_tensor(f"dl{q}", [128, cfg.COLS_Q], F32,
                           kind="ExternalInput") for q in range(cfg.NQ)]
    logits_d = nc.dram_tensor("logits", [128, NT * cfg.N_CLASSES], F32,
                              kind="ExternalOutput")

    rg = [list(range(cfg.NC))]

    with tile.TileContext(nc) as tc:
        with tc.tile_pool(name="const", bufs=1) as cpool, \
             tc.tile_pool(name="dram", bufs=1, space="DRAM") as dpool, \
             tc.tile_pool(name="hp", bufs=3) as hpool:

            hs1_t = dpool.tile([cfg.SLOTS_C, H], F32, tag="hs1")
            tab1_t = dpool.tile([cfg.TABLE_N, H], F32, tag="tab1")
            hs2_t = dpool.tile([cfg.SLOTS_C, H], F32, tag="hs2")
            tab2_t = dpool.tile([cfg.TABLE_N, H], F32, tag="tab2")

            def cload(dram, shape, dt, tag):
                t = cpool.tile(shape, dt, tag=tag)
                nc.sync.dma_start(out=t[:], in_=dram[:, :])
                return t

            w1_s = cload(w1_d, [cfg.IN_FEAT, H], F32, "w1")
            w2_s = cload(w2_d, [H, H], F32, "w2")
            wl_s = cload(wl_d, [H, cfg.N_CLASSES], F32, "wl")
            b1c_s = cload(b1c_d, [H, 1], F32, "b1c")
            b2c_s = cload(b2c_d, [H, 1], F32, "b2c")
            blrep_s = cload(blrep_d, [128, cfg.N_CLASSES], F32, "blrep")
            dinvn_s = cload(dinvn_d, [128, NT], F32, "dinvn")
            dinv2n_s = cload(dinv2n_d, [128, NT], F32, "dinv2n")
            iota_s = cload(iota_d, [128, cfg.BLK], F32, "iota")
            ident_s = cload(ident_d, [128, H], cfg.SELF_DT, "ident")

            self1_s = cpool.tile([128, NT * H], cfg.SELF_DT, tag="self1")
            self2_s = cpool.tile([128, NT * H], cfg.SELF_DT, tag="self2")
            stageL_s = cpool.tile([128, NT * cfg.N_CLASSES], F32, tag="stgL")

            # ---- phase A: table1 = dinv * (x @ W1), plus self terms ----
            with tc.tile_pool(name="xp", bufs=1) as xpool, \
                 tc.tile_pool(name="pA", bufs=2, space="PSUM") as pA:
                xt_s = xpool.tile([cfg.IN_FEAT, cfg.SLOTS_C], F32, tag="xt")
                nc.sync.dma_start(out=xt_s[:], in_=xT_d[:, :])
                for t in range(NT):
                    ps = pA.tile([128, H], F32, tag="a")
                    nc.tensor.matmul(out=ps[:],
                                     lhsT=xt_s[:, t * 128:(t + 1) * 128],
                                     rhs=w1_s[:], start=True, stop=True)
                    row = hpool.tile([128, H], F32, tag="hsrow")
                    nc.vector.tensor_scalar_mul(out=row[:], in0=ps[:],
                                                scalar1=dinvn_s[:, t:t + 1])
                    nc.sync.dma_start(out=hs1_t[t * 128:(t + 1) * 128, :],
                                      in_=row[:])
                    nc.vector.tensor_scalar_mul(
                        out=self1_s[:, t * H:(t + 1) * H], in0=ps[:],
                        scalar1=dinvn_s[:, t:t + 1])

            nc.gpsimd.collective_compute(
                "AllGather", mybir.AluOpType.bypass, replica_groups=rg,
                ins=[hs1_t.opt()], outs=[tab1_t.opt()])

            # ---- phases B (layer1 -> table2) and C (layer2 -> logits) ----
            with tc.tile_pool(name="sp", bufs=2) as spool, \
                 tc.tile_pool(name="pp", bufs=1, space="PSUM") as pp:

                def conv_layer(layer):
                    tab_t = tab1_t if layer == 1 else tab2_t
                    self_s = self1_s if layer == 1 else self2_s
                    bc_s = b1c_s if layer == 1 else b2c_s
                    pair = {}
                    for i in range(cfg.N_BATCH):
                        msgs, Ss = [], []
                        for q in range(cfg.NQ):
                            idx_t = spool.tile([128, cfg.C_BATCH * 8], I16,
                                               tag=f"idx{q}")
                            nc.sync.dma_start(
                                out=idx_t[:],
                                in_=idx_d[q][:, i * cfg.C_BATCH * 8:
                                             (i + 1) * cfg.C_BATCH * 8])
                            dl_t = spool.tile([128, cfg.C_BATCH], F32,
                                              tag=f"dl{q}")
                            nc.sync.dma_start(
                                out=dl_t[:],
                                in_=dl_d[q][:, i * cfg.C_BATCH:
                                            (i + 1) * cfg.C_BATCH])
                            msg_t = spool.tile([128, cfg.C_BATCH, H], F32,
                                               tag=f"msg{q}")
                            nc.gpsimd.dma_gather(
                                out_ap=msg_t[:],
                                in_ap=tab_t[q * cfg.WIN:(q + 1) * cfg.WIN, :],
                                idxs_ap=idx_t[:],
                                num_idxs=cfg.C_BATCH * 128,
                                num_idxs_reg=cfg.C_BATCH * 128,
                                elem_size=H, queue_num=0,
                                single_packet=False)
                            S_t = spool.tile([128, cfg.C_BATCH, cfg.BLK], F32,
                                             tag=f"S{q}")
                            iota_bc = iota_s[:]\
                                .rearrange("p (c f) -> p c f", c=1)\
                                .to_broadcast([128, cfg.C_BATCH, cfg.BLK])
                            dl_bc = dl_t[:]\
                                .rearrange("p (c f) -> p c f", f=1)\
                                .to_broadcast([128, cfg.C_BATCH, cfg.BLK])
                            nc.vector.tensor_tensor(
                                out=S_t[:], in0=iota_bc, in1=dl_bc,
                                op=mybir.AluOpType.is_equal)
                            msgs.append(msg_t[:].rearrange("p c f -> p (c f)"))
                            Ss.append(S_t[:].rearrange("p c f -> p (c f)"))

                        dfm_t = spool.tile([H, cfg.BPB * cfg.BLK], F32,
                                           tag="dfm")
                        nc.sync.dma_start(
                            out=dfm_t[:],
                            in_=dinvfm_d[:, i * cfg.BPB * cfg.BLK:
                                         (i + 1) * cfg.BPB * cfg.BLK])

                        for bb in range(cfg.BPB):
                            b = i * cfg.BPB + bb
                            half = (b % 2) * H
                            t = b // 2
                            pfm = pp.tile([H, cfg.BLK], F32, tag="fm")
                            nc.tensor.matmul(
                                out=pfm[:],
                                lhsT=self_s[half:half + H,
                                            t * H:(t + 1) * H],
                                rhs=ident_s[half:half + H, :],
                                start=True, stop=False)
                            for q in range(cfg.NQ):
                                for k in range(cfg.KCOL):
                                    lc = bb * cfg.KCOL + k
                                    last = (q == cfg.NQ - 1 and
                                            k == cfg.KCOL - 1)
                                    nc.tensor.matmul(
                                        out=pfm[:],
                                        lhsT=msgs[q][:, lc * H:(lc + 1) * H],
                                        rhs=Ss[q][:, lc * cfg.BLK:
                                                  (lc + 1) * cfg.BLK],
                                        start=False, stop=last)
                            h_t = hpool.tile([H, cfg.BLK], F32, tag="h")
                            nc.vector.tensor_tensor(
                                out=h_t[:], in0=pfm[:],
                                in1=dfm_t[:, bb * cfg.BLK:(bb + 1) * cfg.BLK],
                                op=mybir.AluOpType.mult)
                            hr_t = hpool.tile([H, cfg.BLK], F32, tag="hr")
                            nc.scalar.activation(
                                out=hr_t[:], in_=h_t[:],
                                func=mybir.ActivationFunctionType.Relu,
                                bias=bc_s[:])
                            if layer == 1:
                                if b % 2 == 0:
                                    pair["p2"] = pp.tile([128, H], F32, name="p2",
                                                         tag="pair")
                                p2 = pair["p2"]
                                nc.tensor.matmul(
                                    out=p2[half:half + H, :], lhsT=hr_t[:],
                                    rhs=w2_s[:], start=True, stop=True,
                                    tile_position=(0, half))
                                if b % 2 == 1:
                                    row2 = hpool.tile([128, H], F32,
                                                      tag="hs2row")
                                    nc.vector.tensor_scalar_mul(
                                        out=row2[:], in0=p2[:],
                                        scalar1=dinvn_s[:, t:t + 1])
                                    nc.sync.dma_start(
                                        out=hs2_t[t * 128:(t + 1) * 128, :],
                                        in_=row2[:])
                                    nc.vector.tensor_scalar_mul(
                                        out=self2_s[:, t * H:(t + 1) * H],
                                        in0=p2[:],
                                        scalar1=dinvn_s[:, t:t + 1])
                            else:
                                if b % 2 == 0:
                                    pair["pl"] = pp.tile([128, cfg.N_CLASSES],
                                                         F32, name="pl", tag="pl")
                                pl = pair["pl"]
                                nc.tensor.matmul(
                                    out=pl[half:half + H, :], lhsT=hr_t[:],
                                    rhs=wl_s[:], start=True, stop=True,
                                    tile_position=(0, half))
                                if b % 2 == 1:
                                    nCL = cfg.N_CLASSES
                                    nc.vector.tensor_tensor(
                                        out=stageL_s[:, t * nCL:(t + 1) * nCL],
                                        in0=pl[:], in1=blrep_s[:],
                                        op=mybir.AluOpType.add)

                conv_layer(1)
                nc.gpsimd.collective_compute(
                    "AllGather", mybir.AluOpType.bypass, replica_groups=rg,
                    ins=[hs2_t.opt()], outs=[tab2_t.opt()])
                conv_layer(2)

            nc.sync.dma_start(out=logits_d[:, :], in_=stageL_s[:])

    nc.compile()
    return nc


_PROGRAM_CACHE = {}


def get_program(cfg):
    key = id(cfg)
    if key not in _PROGRAM_CACHE:
        _PROGRAM_CACHE[key] = build_program(cfg)
    return _PROGRAM_CACHE[key]


def run(cfg, inputs, trace=False):
    in_maps, node_of_slot = preprocess(cfg, **inputs)
    nc = get_program(cfg)
    res = bass_utils.run_bass_kernel_spmd(
        nc, in_maps, core_ids=list(range(cfg.NC)), trace=trace)
    out = assemble_output(cfg, res.results, node_of_slot)
    return out, res


def kernel(**inputs) -> np.ndarray:
    out, _ = run(CFG_FULL, inputs)
    return out



# revision 3
# speedup vs baseline: 1.7034x; 1.7034x over previous
"""2-layer GCN (GCNConv+relu x2, linear head) on 8 Trainium2 NeuronCores.

Strategy (graph/data parallel, per sharding hint):
  - Nodes sharded across 8 cores by id; edges partitioned by destination.
  - Per core, destination nodes are bin-packed into B_FIX blocks of <=BLK
    dsts such that each (block, source-window) holds <= KCOL*128 edges.
    This gives an SPMD-uniform program; only tensor data varies per core.
  - Per layer: local matmul (x@W scaled by dinv) -> AllGather into a
    full node-major table in DRAM -> dma_gather messages per edge slot
    (4 source windows to satisfy the int16 index range) -> selection
    matrix S built with one batched is_equal DVE op per gather batch ->
    PE matmuls (lhsT=messages, rhs=S) accumulate feature-major conv
    output in PSUM; self-loop terms enter via an identity-matmul
    transpose. Post: scale by dinv, +bias, relu, next-layer matmul.
"""

import numpy as np

import concourse.bass as bass
import concourse.mybir as mybir
import concourse.tile as tile
from concourse import bacc
from concourse import bass_utils

import ml_dtypes

F32 = mybir.dt.float32
BF16 = mybir.dt.bfloat16
I16 = mybir.dt.int16
NP_BF16 = ml_dtypes.bfloat16


class Cfg:
    def __init__(self, n_nodes, in_feat, hidden, n_classes, n_cores, n_c,
                 blk, kcol, b_fix, nq, c_batch, self_dtype="bf16"):
        self.N = n_nodes
        self.IN_FEAT = in_feat
        self.HIDDEN = hidden
        self.N_CLASSES = n_classes
        self.NC = n_cores
        self.N_C = n_c                    # nodes per core (id // N_C)
        assert n_c * n_cores >= n_nodes
        self.BLK = blk                    # max dsts per block
        self.KCOL = kcol                  # columns per (block, stream)
        self.CAP = kcol * 128             # max edges per (block, stream)
        self.B_FIX = b_fix                # blocks per core (uniform)
        self.NQ = nq                      # source windows / gather streams
        self.SLOTS_C = b_fix * blk        # table slots per core
        assert self.SLOTS_C % 128 == 0
        self.NT = self.SLOTS_C // 128     # node tiles per core
        assert self.NT % 2 == 0
        self.TABLE_N = n_cores * self.SLOTS_C
        assert self.TABLE_N % nq == 0
        self.WIN = self.TABLE_N // nq     # table rows per source window
        assert self.WIN <= 32767          # int16 gather index range
        assert (n_cores % nq) == 0
        self.COLS_Q = b_fix * kcol        # gather columns per stream
        self.C_BATCH = c_batch            # columns per gather batch
        assert c_batch % kcol == 0 and self.COLS_Q % c_batch == 0
        self.N_BATCH = self.COLS_Q // c_batch
        self.BPB = c_batch // kcol        # blocks per batch
        assert self.BPB % 2 == 0          # block pairs never straddle batches
        self.SELF_DT = BF16 if self_dtype == "bf16" else F32
        self.NP_SELF = NP_BF16 if self_dtype == "bf16" else np.float32


CFG_FULL = Cfg(n_nodes=100000, in_feat=128, hidden=64, n_classes=16,
               n_cores=8, n_c=12544, blk=64, kcol=2, b_fix=224, nq=4,
               c_batch=28)


# ---------------------------------------------------------------------------
# Host-side preprocessing (sharding): all integer graph restructuring.
# ---------------------------------------------------------------------------

def preprocess(cfg, x, edge_index, W1, b1, W2, b2, Wl, bl):
    N, NC, N_C = cfg.N, cfg.NC, cfg.N_C
    src = np.asarray(edge_index[0]).astype(np.int64)
    dst = np.asarray(edge_index[1]).astype(np.int64)
    x = np.asarray(x, dtype=np.float32)

    deg = np.bincount(dst, minlength=N).astype(np.float32) + 1.0
    dinv = (1.0 / np.sqrt(deg)).astype(np.float32)

    cores_per_q = NC // cfg.NQ
    q_of = (src // N_C) // cores_per_q       # stream of each edge

    # per-(node, q) incoming edge counts
    degq = np.bincount(dst * cfg.NQ + q_of, minlength=N * cfg.NQ)\
             .reshape(N, cfg.NQ)

    # --- per-core first-fit-decreasing packing of dsts into blocks ---
    slot_of = np.full(NC * N_C, -1, dtype=np.int64)
    node_of_slot = np.full(cfg.TABLE_N, -1, dtype=np.int64)
    for c in range(NC):
        lo, hi = c * N_C, min((c + 1) * N_C, N)
        n_here = hi - lo
        if n_here <= 0:
            continue
        dq = degq[lo:hi]
        order = np.argsort(-dq.max(axis=1), kind="stable")
        accs = np.zeros((cfg.B_FIX, cfg.NQ), dtype=np.int64)
        cnts = np.zeros(cfg.B_FIX, dtype=np.int64)
        nopen = 1
        for j in order:
            v = dq[j]
            fits = (cnts[:nopen] < cfg.BLK) & \
                   np.all(accs[:nopen] + v <= cfg.CAP, axis=1)
            w = np.flatnonzero(fits)
            if w.size == 0:
                assert nopen < cfg.B_FIX, \
                    f"core {c}: packing exceeds {cfg.B_FIX} blocks"
                b = nopen
                nopen += 1
            else:
                b = int(w[0])
            g = lo + j
            s = c * cfg.SLOTS_C + b * cfg.BLK + cnts[b]
            slot_of[g] = s
            node_of_slot[s] = g
            accs[b] += v
            cnts[b] += 1

    slot_of = slot_of[:N]

    # --- per-core edge streams ---
    e_core = dst // N_C
    s_slot = slot_of[src]
    d_slot_l = slot_of[dst] - e_core * cfg.SLOTS_C
    e_b = d_slot_l // cfg.BLK
    e_r = d_slot_l % cfg.BLK

    P_Q = cfg.B_FIX * cfg.CAP            # positions per stream
    idx_all = np.zeros((NC, cfg.NQ, P_Q), dtype=np.int16)
    dl_all = np.full((NC, cfg.NQ, P_Q), 255.0, dtype=np.float32)

    order2 = np.lexsort((e_b, q_of, e_core))
    es_c, eq_c, eb_c = e_core[order2], q_of[order2], e_b[order2]
    grp = (es_c * cfg.NQ + eq_c) * cfg.B_FIX + eb_c
    _, start_idx, cnt_grp = np.unique(grp, return_index=True,
                                      return_counts=True)
    rank = np.arange(grp.size) - np.repeat(start_idx, cnt_grp)
    assert rank.max(initial=0) < cfg.CAP
    pos = eb_c * cfg.CAP + rank
    idx_val = (s_slot[order2] % cfg.WIN).astype(np.int16)
    idx_all[es_c, eq_c, pos] = idx_val
    dl_all[es_c, eq_c, pos] = e_r[order2].astype(np.float32)

    # wrapped int16 layout: position i -> [i%16, i//16], replicated x8
    idx_w = idx_all.reshape(NC, cfg.NQ, -1, 16).transpose(0, 1, 3, 2)
    idx_dev = np.ascontiguousarray(np.tile(idx_w, (1, 1, 8, 1)))
    # dstloc layout: position -> [pos%128, pos//128]
    dl_dev = np.ascontiguousarray(
        dl_all.reshape(NC, cfg.NQ, cfg.COLS_Q, 128).transpose(0, 1, 3, 2))

    # --- per-slot node data ---
    valid = node_of_slot >= 0
    xe = np.zeros((cfg.TABLE_N, cfg.IN_FEAT), dtype=np.float32)
    xe[valid] = x[node_of_slot[valid]]
    dinv_s = np.zeros(cfg.TABLE_N, dtype=np.float32)
    dinv_s[valid] = dinv[node_of_slot[valid]]

    W1 = np.asarray(W1, np.float32)
    W2 = np.asarray(W2, np.float32)
    Wl = np.asarray(Wl, np.float32)
    b1 = np.asarray(b1, np.float32)
    b2 = np.asarray(b2, np.float32)
    bl = np.asarray(bl, np.float32)

    iota64 = np.tile(np.arange(cfg.BLK, dtype=np.float32)[None, :], (128, 1))
    ident2 = np.concatenate([np.eye(cfg.HIDDEN), np.eye(cfg.HIDDEN)],
                            axis=0).astype(cfg.NP_SELF)

    in_maps = []
    for c in range(NC):
        sl = slice(c * cfg.SLOTS_C, (c + 1) * cfg.SLOTS_C)
        dv = dinv_s[sl]
        m = {
            "xT": np.ascontiguousarray(xe[sl].T),
            "w1": W1, "w2": W2, "wl": Wl,
            "b1c": b1.reshape(-1, 1), "b2c": b2.reshape(-1, 1),
            "blrep": np.tile(bl[None, :], (128, 1)),
            "dinvn": np.ascontiguousarray(dv.reshape(cfg.NT, 128).T),
            "dinv2n": np.ascontiguousarray((dv * dv).reshape(cfg.NT, 128).T),
            "dinvfm": np.tile(dv[None, :], (cfg.HIDDEN, 1)),
            "iota64": iota64,
            "ident2": ident2,
        }
        for q in range(cfg.NQ):
            m[f"idx{q}"] = idx_dev[c, q]
            m[f"dl{q}"] = dl_dev[c, q]
        in_maps.append(m)

    return in_maps, node_of_slot


def assemble_output(cfg, results, node_of_slot):
    out = np.zeros((cfg.N, cfg.N_CLASSES), dtype=np.float32)
    for c, r in enumerate(results):
        lg = r["logits"].reshape(128, cfg.NT, cfg.N_CLASSES)
        sl = node_of_slot[c * cfg.SLOTS_C:(c + 1) * cfg.SLOTS_C]\
            .reshape(cfg.NT, 128)
        for t in range(cfg.NT):
            v = sl[t] >= 0
            out[sl[t][v]] = lg[v, t, :]
    return out


# ---------------------------------------------------------------------------
# Device program
# ---------------------------------------------------------------------------

def build_program(cfg):
    nc = bacc.Bacc("TRN2", target_bir_lowering=False, debug=False,
                   num_devices=cfg.NC, num_swdge_queues=4)
    H, NT = cfg.HIDDEN, cfg.NT

    xT_d = nc.dram_tensor("xT", [cfg.IN_FEAT, cfg.SLOTS_C], F32,
                          kind="ExternalInput")
    w1_d = nc.dram_tensor("w1", [cfg.IN_FEAT, H], F32, kind="ExternalInput")
    w2_d = nc.dram_tensor("w2", [H, H], F32, kind="ExternalInput")
    wl_d = nc.dram_tensor("wl", [H, cfg.N_CLASSES], F32, kind="ExternalInput")
    b1c_d = nc.dram_tensor("b1c", [H, 1], F32, kind="ExternalInput")
    b2c_d = nc.dram_tensor("b2c", [H, 1], F32, kind="ExternalInput")
    blrep_d = nc.dram_tensor("blrep", [128, cfg.N_CLASSES], F32,
                             kind="ExternalInput")
    dinvn_d = nc.dram_tensor("dinvn", [128, NT], F32, kind="ExternalInput")
    dinv2n_d = nc.dram_tensor("dinv2n", [128, NT], F32, kind="ExternalInput")
    dinvfm_d = nc.dram_tensor("dinvfm", [H, cfg.SLOTS_C], F32,
                              kind="ExternalInput")
    iota_d = nc.dram_tensor("iota64", [128, cfg.BLK], F32,
                            kind="ExternalInput")
    ident_d = nc.dram_tensor("ident2", [128, H], cfg.SELF_DT,
                             kind="ExternalInput")
    idx_d = [nc.dram_tensor(f"idx{q}", [128, cfg.COLS_Q * 8], I16,
                            kind="ExternalInput") for q in range(cfg.NQ)]
    dl_d = [nc.dram_tensor(f"dl{q}", [128, cfg.COLS_Q], F32,
                           kind="ExternalInput") for q in range(cfg.NQ)]
    logits_d = nc.dram_tensor("logits", [128, NT * cfg.N_CLASSES], F32,
                              kind="ExternalOutput")

    rg = [list(range(cfg.NC))]

    with tile.TileContext(nc) as tc:
        with tc.tile_pool(name="const", bufs=1) as cpool, \
             tc.tile_pool(name="dram", bufs=1, space="DRAM") as dpool, \
             tc.tile_pool(name="hp", bufs=3) as hpool:

            hs1_t = dpool.tile([cfg.SLOTS_C, H], F32, tag="hs1")
            tab1_t = dpool.tile([cfg.TABLE_N, H], F32, tag="tab1")
            hs2_t = dpool.tile([cfg.SLOTS_C, H], F32, tag="hs2")
            tab2_t = dpool.tile([cfg.TABLE_N, H], F32, tag="tab2")

            def cload(dram, shape, dt, tag):
                t = cpool.tile(shape, dt, tag=tag)
                nc.sync.dma_start(out=t[:], in_=dram[:, :])
                return t

            w1_s = cload(w1_d, [cfg.IN_FEAT, H], F32, "w1")
            w2_s = cload(w2_d, [H, H], F32, "w2")
            wl_s = cload(wl_d, [H, cfg.N_CLASSES], F32, "wl")
            b1c_s = cload(b1c_d, [H, 1], F32, "b1c")
            b2c_s = cload(b2c_d, [H, 1], F32, "b2c")
            blrep_s = cload(blrep_d, [128, cfg.N_CLASSES], F32, "blrep")
            dinvn_s = cload(dinvn_d, [128, NT], F32, "dinvn")
            dinv2n_s = cload(dinv2n_d, [128, NT], F32, "dinv2n")
            iota_s = cload(iota_d, [128, cfg.BLK], F32, "iota")
            ident_s = cload(ident_d, [128, H], cfg.SELF_DT, "ident")

            self1_s = cpool.tile([128, NT * H], cfg.SELF_DT, tag="self1")
            self2_s = cpool.tile([128, NT * H], cfg.SELF_DT, tag="self2")
            stageL_s = cpool.tile([128, NT * cfg.N_CLASSES], F32, tag="stgL")

            # ---- phase A: table1 = dinv * (x @ W1), plus self terms ----
            with tc.tile_pool(name="xp", bufs=1) as xpool, \
                 tc.tile_pool(name="pA", bufs=2, space="PSUM") as pA:
                xt_s = xpool.tile([cfg.IN_FEAT, cfg.SLOTS_C], F32, tag="xt")
                nc.sync.dma_start(out=xt_s[:], in_=xT_d[:, :])
                for t in range(NT):
                    ps = pA.tile([128, H], F32, tag="a")
                    nc.tensor.matmul(out=ps[:],
                                     lhsT=xt_s[:, t * 128:(t + 1) * 128],
                                     rhs=w1_s[:], start=True, stop=True)
                    row = hpool.tile([128, H], F32, tag="hsrow")
                    nc.vector.tensor_scalar_mul(out=row[:], in0=ps[:],
                                                scalar1=dinvn_s[:, t:t + 1])
                    nc.sync.dma_start(out=hs1_t[t * 128:(t + 1) * 128, :],
                                      in_=row[:])
                    nc.vector.tensor_scalar_mul(
                        out=self1_s[:, t * H:(t + 1) * H], in0=ps[:],
                        scalar1=dinvn_s[:, t:t + 1])

            nc.gpsimd.collective_compute(
                "AllGather", mybir.AluOpType.bypass, replica_groups=rg,
                ins=[hs1_t.opt()], outs=[tab1_t.opt()])

            # ---- phases B (layer1 -> table2) and C (layer2 -> logits) ----
            with tc.tile_pool(name="sp", bufs=2) as spool, \
                 tc.tile_pool(name="pp", bufs=1, space="PSUM") as pp:

                def conv_layer(layer):
                    tab_t = tab1_t if layer == 1 else tab2_t
                    self_s = self1_s if layer == 1 else self2_s
                    bc_s = b1c_s if layer == 1 else b2c_s
                    pair = {}
                    for i in range(cfg.N_BATCH):
                        msgs, Ss = [], []
                        for q in range(cfg.NQ):
                            idx_t = spool.tile([128, cfg.C_BATCH * 8], I16,
                                               tag=f"idx{q}")
                            nc.sync.dma_start(
                                out=idx_t[:],
                                in_=idx_d[q][:, i * cfg.C_BATCH * 8:
                                             (i + 1) * cfg.C_BATCH * 8])
                            dl_t = spool.tile([128, cfg.C_BATCH], F32,
                                              tag=f"dl{q}")
                            nc.sync.dma_start(
                                out=dl_t[:],
                                in_=dl_d[q][:, i * cfg.C_BATCH:
                                            (i + 1) * cfg.C_BATCH])
                            msg_t = spool.tile([128, cfg.C_BATCH, H], F32,
                                               tag=f"msg{q}")
                            nc.gpsimd.dma_gather(
                                out_ap=msg_t[:],
                                in_ap=tab_t[q * cfg.WIN:(q + 1) * cfg.WIN, :],
                                idxs_ap=idx_t[:],
                                num_idxs=cfg.C_BATCH * 128,
                                num_idxs_reg=cfg.C_BATCH * 128,
                                elem_size=H, queue_num=q,
                                single_packet=False)
                            S_t = spool.tile([128, cfg.C_BATCH, cfg.BLK], F32,
                                             tag=f"S{q}")
                            iota_bc = iota_s[:]\
                                .rearrange("p (c f) -> p c f", c=1)\
                                .to_broadcast([128, cfg.C_BATCH, cfg.BLK])
                            dl_bc = dl_t[:]\
                                .rearrange("p (c f) -> p c f", f=1)\
                                .to_broadcast([128, cfg.C_BATCH, cfg.BLK])
                            nc.vector.tensor_tensor(
                                out=S_t[:], in0=iota_bc, in1=dl_bc,
                                op=mybir.AluOpType.is_equal)
                            msgs.append(msg_t[:].rearrange("p c f -> p (c f)"))
                            Ss.append(S_t[:].rearrange("p c f -> p (c f)"))

                        dfm_t = spool.tile([H, cfg.BPB * cfg.BLK], F32,
                                           tag="dfm")
                        nc.sync.dma_start(
                            out=dfm_t[:],
                            in_=dinvfm_d[:, i * cfg.BPB * cfg.BLK:
                                         (i + 1) * cfg.BPB * cfg.BLK])

                        for bb in range(cfg.BPB):
                            b = i * cfg.BPB + bb
                            half = (b % 2) * H
                            t = b // 2
                            pfm = pp.tile([H, cfg.BLK], F32, tag="fm")
                            nc.tensor.matmul(
                                out=pfm[:],
                                lhsT=self_s[half:half + H,
                                            t * H:(t + 1) * H],
                                rhs=ident_s[half:half + H, :],
                                start=True, stop=False)
                            for q in range(cfg.NQ):
                                for k in range(cfg.KCOL):
                                    lc = bb * cfg.KCOL + k
                                    last = (q == cfg.NQ - 1 and
                                            k == cfg.KCOL - 1)
                                    nc.tensor.matmul(
                                        out=pfm[:],
                                        lhsT=msgs[q][:, lc * H:(lc + 1) * H],
                                        rhs=Ss[q][:, lc * cfg.BLK:
                                                  (lc + 1) * cfg.BLK],
                                        start=False, stop=last)
                            h_t = hpool.tile([H, cfg.BLK], F32, tag="h")
                            nc.vector.tensor_tensor(
                                out=h_t[:], in0=pfm[:],
                                in1=dfm_t[:, bb * cfg.BLK:(bb + 1) * cfg.BLK],
                                op=mybir.AluOpType.mult)
                            hr_t = hpool.tile([H, cfg.BLK], F32, tag="hr")
                            nc.scalar.activation(
                                out=hr_t[:], in_=h_t[:],
                                func=mybir.ActivationFunctionType.Relu,
                                bias=bc_s[:])
                            if layer == 1:
                                if b % 2 == 0:
                                    pair["p2"] = pp.tile([128, H], F32, name="p2",
                                                         tag="pair")
                                p2 = pair["p2"]
                                nc.tensor.matmul(
                                    out=p2[half:half + H, :], lhsT=hr_t[:],
                                    rhs=w2_s[:], start=True, stop=True,
                                    tile_position=(0, half))
                                if b % 2 == 1:
                                    row2 = hpool.tile([128, H], F32,
                                                      tag="hs2row")
                                    nc.vector.tensor_scalar_mul(
                                        out=row2[:], in0=p2[:],
                                        scalar1=dinvn_s[:, t:t + 1])
                                    nc.sync.dma_start(
                                        out=hs2_t[t * 128:(t + 1) * 128, :],
                                        in_=row2[:])
                                    nc.vector.tensor_scalar_mul(
                                        out=self2_s[:, t * H:(t + 1) * H],
                                        in0=p2[:],
                                        scalar1=dinvn_s[:, t:t + 1])
                            else:
                                if b % 2 == 0:
                                    pair["pl"] = pp.tile([128, cfg.N_CLASSES],
                                                         F32, name="pl", tag="pl")
                                pl = pair["pl"]
                                nc.tensor.matmul(
                                    out=pl[half:half + H, :], lhsT=hr_t[:],
                                    rhs=wl_s[:], start=True, stop=True,
                                    tile_position=(0, half))
                                if b % 2 == 1:
                                    nCL = cfg.N_CLASSES
                                    nc.vector.tensor_tensor(
                                        out=stageL_s[:, t * nCL:(t + 1) * nCL],
                                        in0=pl[:], in1=blrep_s[:],
                                        op=mybir.AluOpType.add)

                conv_layer(1)
                nc.gpsimd.collective_compute(
                    "AllGather", mybir.AluOpType.bypass, replica_groups=rg,
                    ins=[hs2_t.opt()], outs=[tab2_t.opt()])
                conv_layer(2)

            nc.sync.dma_start(out=logits_d[:, :], in_=stageL_s[:])

    nc.compile()
    return nc


_PROGRAM_CACHE = {}


def get_program(cfg):
    key = id(cfg)
    if key not in _PROGRAM_CACHE:
        _PROGRAM_CACHE[key] = build_program(cfg)
    return _PROGRAM_CACHE[key]


def run(cfg, inputs, trace=False):
    in_maps, node_of_slot = preprocess(cfg, **inputs)
    nc = get_program(cfg)
    res = bass_utils.run_bass_kernel_spmd(
        nc, in_maps, core_ids=list(range(cfg.NC)), trace=trace)
    out = assemble_output(cfg, res.results, node_of_slot)
    return out, res


def kernel(**inputs) -> np.ndarray:
    out, _ = run(CFG_FULL, inputs)
    return out



# revision 11
# speedup vs baseline: 2.0929x; 1.2287x over previous
"""2-layer GCN (GCNConv+relu x2, linear head) on 8 Trainium2 NeuronCores.

Strategy (graph/data parallel, per sharding hint):
  - Nodes sharded across 8 cores by id; edges partitioned by destination.
  - Per core, destination nodes are bin-packed into B_FIX blocks of <=BLK
    dsts such that each (block, source-window) holds <= KCOL*128 edges.
    This gives an SPMD-uniform program; only tensor data varies per core.
  - Per layer: local matmul (x@W scaled by dinv) -> AllGather into a
    full node-major bf16 table in DRAM -> dma_gather one 256B element
    per edge = a PAIR of bf16 rows (slots 2w, 2w+1); parity-split
    selection matrices S_even/S_odd (is_equal on DVE, bf16) route the
    correct half; PE bf16 matmuls accumulate feature-major conv output
    in PSUM; self-loop terms enter via an identity-matmul transpose.
    Post: scale by dinv, +bias, relu, next-layer matmul (bf16).
  - The 4 source-window gathers go to 4 SWDGE queues: each queue's
    descriptor generation runs on its own GpSimd Q7 core pair, so the
    4 gathers of a batch overlap (queue 0 issued last since its pair
    is the one the engine timeline blocks on).
"""

import numpy as np

import concourse.bass as bass
import concourse.mybir as mybir
import concourse.tile as tile
from concourse import bacc
from concourse import bass_utils

import ml_dtypes

F32 = mybir.dt.float32
BF16 = mybir.dt.bfloat16
I16 = mybir.dt.int16
NP_BF16 = ml_dtypes.bfloat16


class Cfg:
    def __init__(self, n_nodes, in_feat, hidden, n_classes, n_cores, n_c,
                 blk, kcol, b_fix, nq, c_batch, self_dtype="bf16"):
        self.N = n_nodes
        self.IN_FEAT = in_feat
        self.HIDDEN = hidden
        self.N_CLASSES = n_classes
        self.NC = n_cores
        self.N_C = n_c                    # nodes per core (id // N_C)
        assert n_c * n_cores >= n_nodes
        self.BLK = blk                    # max dsts per block
        self.KCOL = kcol                  # columns per (block, stream)
        self.CAP = kcol * 128             # max edges per (block, stream)
        self.B_FIX = b_fix                # blocks per core (uniform)
        self.NQ = nq                      # source windows / gather streams
        self.SLOTS_C = b_fix * blk        # table slots per core
        assert self.SLOTS_C % 128 == 0
        self.NT = self.SLOTS_C // 128     # node tiles per core
        assert self.NT % 2 == 0
        self.TABLE_N = n_cores * self.SLOTS_C
        assert self.TABLE_N % nq == 0
        self.WIN = self.TABLE_N // nq     # table rows per source window
        assert self.WIN <= 32767          # int16 gather index range
        assert (n_cores % nq) == 0
        self.COLS_Q = b_fix * kcol        # gather columns per stream
        self.C_BATCH = c_batch            # columns per gather batch
        assert c_batch % kcol == 0 and self.COLS_Q % c_batch == 0
        self.N_BATCH = self.COLS_Q // c_batch
        self.BPB = c_batch // kcol        # blocks per batch
        assert self.BPB % 2 == 0          # block pairs never straddle batches
        self.SELF_DT = BF16 if self_dtype == "bf16" else F32
        self.NP_SELF = NP_BF16 if self_dtype == "bf16" else np.float32


CFG_FULL = Cfg(n_nodes=100000, in_feat=128, hidden=64, n_classes=16,
               n_cores=8, n_c=12544, blk=64, kcol=2, b_fix=224, nq=4,
               c_batch=28)


# ---------------------------------------------------------------------------
# Host-side preprocessing (sharding): all integer graph restructuring.
# ---------------------------------------------------------------------------

def preprocess(cfg, x, edge_index, W1, b1, W2, b2, Wl, bl):
    N, NC, N_C = cfg.N, cfg.NC, cfg.N_C
    src = np.asarray(edge_index[0]).astype(np.int64)
    dst = np.asarray(edge_index[1]).astype(np.int64)
    x = np.asarray(x, dtype=np.float32)

    deg = np.bincount(dst, minlength=N).astype(np.float32) + 1.0
    dinv = (1.0 / np.sqrt(deg)).astype(np.float32)

    cores_per_q = NC // cfg.NQ
    q_of = (src // N_C) // cores_per_q       # stream of each edge

    # per-(node, q) incoming edge counts
    degq = np.bincount(dst * cfg.NQ + q_of, minlength=N * cfg.NQ)\
             .reshape(N, cfg.NQ)

    # --- per-core first-fit-decreasing packing of dsts into blocks ---
    slot_of = np.full(NC * N_C, -1, dtype=np.int64)
    node_of_slot = np.full(cfg.TABLE_N, -1, dtype=np.int64)
    for c in range(NC):
        lo, hi = c * N_C, min((c + 1) * N_C, N)
        n_here = hi - lo
        if n_here <= 0:
            continue
        dq = degq[lo:hi]
        order = np.argsort(-dq.max(axis=1), kind="stable")
        accs = np.zeros((cfg.B_FIX, cfg.NQ), dtype=np.int64)
        cnts = np.zeros(cfg.B_FIX, dtype=np.int64)
        nopen = 1
        for j in order:
            v = dq[j]
            fits = (cnts[:nopen] < cfg.BLK) & \
                   np.all(accs[:nopen] + v <= cfg.CAP, axis=1)
            w = np.flatnonzero(fits)
            if w.size == 0:
                assert nopen < cfg.B_FIX, \
                    f"core {c}: packing exceeds {cfg.B_FIX} blocks"
                b = nopen
                nopen += 1
            else:
                b = int(w[0])
            g = lo + j
            s = c * cfg.SLOTS_C + b * cfg.BLK + cnts[b]
            slot_of[g] = s
            node_of_slot[s] = g
            accs[b] += v
            cnts[b] += 1

    slot_of = slot_of[:N]

    # --- per-core edge streams ---
    e_core = dst // N_C
    s_slot = slot_of[src]
    d_slot_l = slot_of[dst] - e_core * cfg.SLOTS_C
    e_b = d_slot_l // cfg.BLK
    e_r = d_slot_l % cfg.BLK

    P_Q = cfg.B_FIX * cfg.CAP            # positions per stream
    idx_all = np.zeros((NC, cfg.NQ, P_Q), dtype=np.int16)
    # parity-split dst-row tables: 255 = inactive
    dle_all = np.full((NC, cfg.NQ, P_Q), 255.0, dtype=np.float32)
    dlo_all = np.full((NC, cfg.NQ, P_Q), 255.0, dtype=np.float32)

    order2 = np.lexsort((e_b, q_of, e_core))
    es_c, eq_c, eb_c = e_core[order2], q_of[order2], e_b[order2]
    grp = (es_c * cfg.NQ + eq_c) * cfg.B_FIX + eb_c
    _, start_idx, cnt_grp = np.unique(grp, return_index=True,
                                      return_counts=True)
    rank = np.arange(grp.size) - np.repeat(start_idx, cnt_grp)
    assert rank.max(initial=0) < cfg.CAP
    pos = eb_c * cfg.CAP + rank
    s_sorted = s_slot[order2]
    # gather PAIR index (two table rows per 256B element)
    idx_val = ((s_sorted % cfg.WIN) // 2).astype(np.int16)
    idx_all[es_c, eq_c, pos] = idx_val
    par = (s_sorted % 2).astype(np.int64)
    er_f = e_r[order2].astype(np.float32)
    even = par == 0
    dle_all[es_c[even], eq_c[even], pos[even]] = er_f[even]
    dlo_all[es_c[~even], eq_c[~even], pos[~even]] = er_f[~even]

    # wrapped int16 layout: position i -> [i%16, i//16], replicated x8
    idx_w = idx_all.reshape(NC, cfg.NQ, -1, 16).transpose(0, 1, 3, 2)
    idx_dev = np.ascontiguousarray(np.tile(idx_w, (1, 1, 8, 1)))
    # dstloc layout: position -> [pos%128, pos//128], bf16
    dle_dev = np.ascontiguousarray(
        dle_all.reshape(NC, cfg.NQ, cfg.COLS_Q, 128).transpose(0, 1, 3, 2)
    ).astype(NP_BF16)
    dlo_dev = np.ascontiguousarray(
        dlo_all.reshape(NC, cfg.NQ, cfg.COLS_Q, 128).transpose(0, 1, 3, 2)
    ).astype(NP_BF16)

    # --- per-slot node data ---
    valid = node_of_slot >= 0
    xe = np.zeros((cfg.TABLE_N, cfg.IN_FEAT), dtype=np.float32)
    xe[valid] = x[node_of_slot[valid]]
    dinv_s = np.zeros(cfg.TABLE_N, dtype=np.float32)
    dinv_s[valid] = dinv[node_of_slot[valid]]

    W1 = np.asarray(W1, np.float32)
    W2 = np.asarray(W2, np.float32).astype(NP_BF16)
    Wl = np.asarray(Wl, np.float32).astype(NP_BF16)
    b1 = np.asarray(b1, np.float32)
    b2 = np.asarray(b2, np.float32)
    bl = np.asarray(bl, np.float32)

    iota64 = np.tile(np.arange(cfg.BLK, dtype=np.float32)[None, :],
                     (128, 1)).astype(NP_BF16)
    ident2 = np.concatenate([np.eye(cfg.HIDDEN), np.eye(cfg.HIDDEN)],
                            axis=0).astype(cfg.NP_SELF)

    in_maps = []
    for c in range(NC):
        sl = slice(c * cfg.SLOTS_C, (c + 1) * cfg.SLOTS_C)
        dv = dinv_s[sl]
        m = {
            "xT": np.ascontiguousarray(xe[sl].T),
            "w1": W1, "w2": W2, "wl": Wl,
            "b1c": b1.reshape(-1, 1), "b2c": b2.reshape(-1, 1),
            "blrep": np.tile(bl[None, :], (128, 1)),
            "dinvn": np.ascontiguousarray(dv.reshape(cfg.NT, 128).T),
            "dinvfm": np.tile(dv[None, :], (cfg.HIDDEN, 1)),
            "iota64": iota64,
            "ident2": ident2,
        }
        for q in range(cfg.NQ):
            m[f"idx{q}"] = idx_dev[c, q]
            m[f"dle{q}"] = dle_dev[c, q]
            m[f"dlo{q}"] = dlo_dev[c, q]
        in_maps.append(m)

    return in_maps, node_of_slot


def assemble_output(cfg, results, node_of_slot):
    out = np.zeros((cfg.N, cfg.N_CLASSES), dtype=np.float32)
    for c, r in enumerate(results):
        lg = r["logits"].reshape(128, cfg.NT, cfg.N_CLASSES)
        sl = node_of_slot[c * cfg.SLOTS_C:(c + 1) * cfg.SLOTS_C]\
            .reshape(cfg.NT, 128)
        for t in range(cfg.NT):
            v = sl[t] >= 0
            out[sl[t][v]] = lg[v, t, :]
    return out


# ---------------------------------------------------------------------------
# Device program
# ---------------------------------------------------------------------------

def build_program(cfg):
    nc = bacc.Bacc("TRN2", target_bir_lowering=False, debug=False,
                   num_devices=cfg.NC, num_swdge_queues=4)
    H, NT = cfg.HIDDEN, cfg.NT

    xT_d = nc.dram_tensor("xT", [cfg.IN_FEAT, cfg.SLOTS_C], F32,
                          kind="ExternalInput")
    w1_d = nc.dram_tensor("w1", [cfg.IN_FEAT, H], F32, kind="ExternalInput")
    w2_d = nc.dram_tensor("w2", [H, H], BF16, kind="ExternalInput")
    wl_d = nc.dram_tensor("wl", [H, cfg.N_CLASSES], BF16,
                          kind="ExternalInput")
    b1c_d = nc.dram_tensor("b1c", [H, 1], F32, kind="ExternalInput")
    b2c_d = nc.dram_tensor("b2c", [H, 1], F32, kind="ExternalInput")
    blrep_d = nc.dram_tensor("blrep", [128, cfg.N_CLASSES], F32,
                             kind="ExternalInput")
    dinvn_d = nc.dram_tensor("dinvn", [128, NT], F32, kind="ExternalInput")
    dinvfm_d = nc.dram_tensor("dinvfm", [H, cfg.SLOTS_C], F32,
                              kind="ExternalInput")
    iota_d = nc.dram_tensor("iota64", [128, cfg.BLK], BF16,
                            kind="ExternalInput")
    ident_d = nc.dram_tensor("ident2", [128, H], cfg.SELF_DT,
                             kind="ExternalInput")
    idx_d = [nc.dram_tensor(f"idx{q}", [128, cfg.COLS_Q * 8], I16,
                            kind="ExternalInput") for q in range(cfg.NQ)]
    dle_d = [nc.dram_tensor(f"dle{q}", [128, cfg.COLS_Q], BF16,
                            kind="ExternalInput") for q in range(cfg.NQ)]
    dlo_d = [nc.dram_tensor(f"dlo{q}", [128, cfg.COLS_Q], BF16,
                            kind="ExternalInput") for q in range(cfg.NQ)]
    logits_d = nc.dram_tensor("logits", [128, NT * cfg.N_CLASSES], F32,
                              kind="ExternalOutput")

    rg = [list(range(cfg.NC))]

    with tile.TileContext(nc) as tc:
        with tc.tile_pool(name="const", bufs=1) as cpool, \
             tc.tile_pool(name="dram", bufs=1, space="DRAM") as dpool, \
             tc.tile_pool(name="hp", bufs=3) as hpool:

            hs1_t = dpool.tile([cfg.SLOTS_C, H], BF16, tag="hs1")
            tab1_t = dpool.tile([cfg.TABLE_N, H], BF16, tag="tab1")
            hs2_t = dpool.tile([cfg.SLOTS_C, H], BF16, tag="hs2")
            tab2_t = dpool.tile([cfg.TABLE_N, H], BF16, tag="tab2")

            def cload(dram, shape, dt, tag):
                t = cpool.tile(shape, dt, tag=tag)
                nc.sync.dma_start(out=t[:], in_=dram[:, :])
                return t

            w1_s = cload(w1_d, [cfg.IN_FEAT, H], F32, "w1")
            w2_s = cload(w2_d, [H, H], BF16, "w2")
            wl_s = cload(wl_d, [H, cfg.N_CLASSES], BF16, "wl")
            b1c_s = cload(b1c_d, [H, 1], F32, "b1c")
            b2c_s = cload(b2c_d, [H, 1], F32, "b2c")
            blrep_s = cload(blrep_d, [128, cfg.N_CLASSES], F32, "blrep")
            dinvn_s = cload(dinvn_d, [128, NT], F32, "dinvn")
            iota_s = cload(iota_d, [128, cfg.BLK], BF16, "iota")
            ident_s = cload(ident_d, [128, H], cfg.SELF_DT, "ident")

            self1_s = cpool.tile([128, NT * H], cfg.SELF_DT, tag="self1")
            self2_s = cpool.tile([128, NT * H], cfg.SELF_DT, tag="self2")
            stageL_s = cpool.tile([128, NT * cfg.N_CLASSES], F32, tag="stgL")

            # ---- phase A: table1 = dinv * (x @ W1), plus self terms ----
            with tc.tile_pool(name="xp", bufs=1) as xpool, \
                 tc.tile_pool(name="pA", bufs=2, space="PSUM") as pA:
                xt_s = xpool.tile([cfg.IN_FEAT, cfg.SLOTS_C], F32, tag="xt")
                nc.sync.dma_start(out=xt_s[:], in_=xT_d[:, :])
                for t in range(NT):
                    ps = pA.tile([128, H], F32, tag="a")
                    nc.tensor.matmul(out=ps[:],
                                     lhsT=xt_s[:, t * 128:(t + 1) * 128],
                                     rhs=w1_s[:], start=True, stop=True)
                    row = hpool.tile([128, H], BF16, tag="hsrow")
                    nc.vector.tensor_scalar_mul(out=row[:], in0=ps[:],
                                                scalar1=dinvn_s[:, t:t + 1])
                    nc.sync.dma_start(out=hs1_t[t * 128:(t + 1) * 128, :],
                                      in_=row[:])
                    nc.vector.tensor_scalar_mul(
                        out=self1_s[:, t * H:(t + 1) * H], in0=ps[:],
                        scalar1=dinvn_s[:, t:t + 1])

            nc.gpsimd.collective_compute(
                "AllGather", mybir.AluOpType.bypass, replica_groups=rg,
                ins=[hs1_t.opt()], outs=[tab1_t.opt()])

            # ---- phases B (layer1 -> table2) and C (layer2 -> logits) ----
            with tc.tile_pool(name="sp", bufs=2) as spool, \
                 tc.tile_pool(name="pp", bufs=1, space="PSUM") as pp:

                def conv_layer(layer):
                    tab_t = tab1_t if layer == 1 else tab2_t
                    self_s = self1_s if layer == 1 else self2_s
                    bc_s = b1c_s if layer == 1 else b2c_s
                    # paired-row view of the table: one 256B gather element
                    # covers two consecutive bf16 rows (slots 2w, 2w+1)
                    tabp = tab_t[:].rearrange("(n two) h -> n (two h)", two=2)
                    pair = {}
                    for i in range(cfg.N_BATCH):
                        msgs, Se, So = [None] * cfg.NQ, [None] * cfg.NQ, \
                            [None] * cfg.NQ
                        for q in (list(range(1, cfg.NQ)) + [0]):
                            idx_t = spool.tile([128, cfg.C_BATCH * 8], I16,
                                               tag=f"idx{q}")
                            nc.sync.dma_start(
                                out=idx_t[:],
                                in_=idx_d[q][:, i * cfg.C_BATCH * 8:
                                             (i + 1) * cfg.C_BATCH * 8])
                            msg_t = spool.tile([128, cfg.C_BATCH, 2 * H],
                                               BF16, tag=f"msg{q}")
                            nc.gpsimd.dma_gather(
                                out_ap=msg_t[:],
                                in_ap=tabp[q * cfg.WIN // 2:
                                           (q + 1) * cfg.WIN // 2, :],
                                idxs_ap=idx_t[:],
                                num_idxs=cfg.C_BATCH * 128,
                                num_idxs_reg=cfg.C_BATCH * 128,
                                elem_size=2 * H, queue_num=q,
                                single_packet=False)
                            msgs[q] = msg_t[:].rearrange("p c f -> p (c f)")
                        for q in range(cfg.NQ):
                            for par, dl_d_, S_lst in ((0, dle_d, Se),
                                                      (1, dlo_d, So)):
                                dl_t = spool.tile([128, cfg.C_BATCH], BF16,
                                                  tag=f"dl{par}_{q}")
                                nc.sync.dma_start(
                                    out=dl_t[:],
                                    in_=dl_d_[q][:, i * cfg.C_BATCH:
                                                 (i + 1) * cfg.C_BATCH])
                                S_t = spool.tile(
                                    [128, cfg.C_BATCH, cfg.BLK], BF16,
                                    tag=f"S{par}_{q}")
                                iota_bc = iota_s[:]\
                                    .rearrange("p (c f) -> p c f", c=1)\
                                    .to_broadcast([128, cfg.C_BATCH, cfg.BLK])
                                dl_bc = dl_t[:]\
                                    .rearrange("p (c f) -> p c f", f=1)\
                                    .to_broadcast([128, cfg.C_BATCH, cfg.BLK])
                                nc.vector.tensor_tensor(
                                    out=S_t[:], in0=iota_bc, in1=dl_bc,
                                    op=mybir.AluOpType.is_equal)
                                S_lst[q] = S_t[:].rearrange(
                                    "p c f -> p (c f)")

                        dfm_t = spool.tile([H, cfg.BPB * cfg.BLK], F32,
                                           tag="dfm")
                        nc.sync.dma_start(
                            out=dfm_t[:],
                            in_=dinvfm_d[:, i * cfg.BPB * cfg.BLK:
                                         (i + 1) * cfg.BPB * cfg.BLK])

                        for bb in range(cfg.BPB):
                            b = i * cfg.BPB + bb
                            half = (b % 2) * H
                            t = b // 2
                            pfm = pp.tile([H, cfg.BLK], F32, tag="fm")
                            nc.tensor.matmul(
                                out=pfm[:],
                                lhsT=self_s[half:half + H,
                                            t * H:(t + 1) * H],
                                rhs=ident_s[half:half + H, :],
                                start=True, stop=False)
                            for q in range(cfg.NQ):
                                for k in range(cfg.KCOL):
                                    lc = bb * cfg.KCOL + k
                                    last = (q == cfg.NQ - 1 and
                                            k == cfg.KCOL - 1)
                                    nc.tensor.matmul(
                                        out=pfm[:],
                                        lhsT=msgs[q][:, lc * 2 * H:
                                                     lc * 2 * H + H],
                                        rhs=Se[q][:, lc * cfg.BLK:
                                                  (lc + 1) * cfg.BLK],
                                        start=False, stop=False)
                                    nc.tensor.matmul(
                                        out=pfm[:],
                                        lhsT=msgs[q][:, lc * 2 * H + H:
                                                     (lc + 1) * 2 * H],
                                        rhs=So[q][:, lc * cfg.BLK:
                                                  (lc + 1) * cfg.BLK],
                                        start=False, stop=last)
                            h_t = hpool.tile([H, cfg.BLK], F32, tag="h")
                            nc.vector.tensor_tensor(
                                out=h_t[:], in0=pfm[:],
                                in1=dfm_t[:, bb * cfg.BLK:(bb + 1) * cfg.BLK],
                                op=mybir.AluOpType.mult)
                            hr_t = hpool.tile([H, cfg.BLK], BF16, tag="hr")
                            nc.scalar.activation(
                                out=hr_t[:], in_=h_t[:],
                                func=mybir.ActivationFunctionType.Relu,
                                bias=bc_s[:])
                            if layer == 1:
                                if b % 2 == 0:
                                    pair["p2"] = pp.tile([128, H], F32, name="p2",
                                                         tag="pair")
                                p2 = pair["p2"]
                                nc.tensor.matmul(
                                    out=p2[half:half + H, :], lhsT=hr_t[:],
                                    rhs=w2_s[:], start=True, stop=True,
                                    tile_position=(0, half))
                                if b % 2 == 1:
                                    row2 = hpool.tile([128, H], BF16,
                                                      tag="hs2row")
                                    nc.vector.tensor_scalar_mul(
                                        out=row2[:], in0=p2[:],
                                        scalar1=dinvn_s[:, t:t + 1])
                                    nc.sync.dma_start(
                                        out=hs2_t[t * 128:(t + 1) * 128, :],
                                        in_=row2[:])
                                    nc.vector.tensor_scalar_mul(
                                        out=self2_s[:, t * H:(t + 1) * H],
                                        in0=p2[:],
                                        scalar1=dinvn_s[:, t:t + 1])
                            else:
                                if b % 2 == 0:
                                    pair["pl"] = pp.tile([128, cfg.N_CLASSES],
                                                         F32, name="pl", tag="pl")
                                pl = pair["pl"]
                                nc.tensor.matmul(
                                    out=pl[half:half + H, :], lhsT=hr_t[:],
                                    rhs=wl_s[:], start=True, stop=True,
                                    tile_position=(0, half))
                                if b % 2 == 1:
                                    nCL = cfg.N_CLASSES
                                    nc.vector.tensor_tensor(
                                        out=stageL_s[:, t * nCL:(t + 1) * nCL],
                                        in0=pl[:], in1=blrep_s[:],
                                        op=mybir.AluOpType.add)

                conv_layer(1)
                nc.gpsimd.collective_compute(
                    "AllGather", mybir.AluOpType.bypass, replica_groups=rg,
                    ins=[hs2_t.opt()], outs=[tab2_t.opt()])
                conv_layer(2)

            nc.sync.dma_start(out=logits_d[:, :], in_=stageL_s[:])

    nc.compile()
    return nc


_PROGRAM_CACHE = {}


def get_program(cfg):
    key = id(cfg)
    if key not in _PROGRAM_CACHE:
        _PROGRAM_CACHE[key] = build_program(cfg)
    return _PROGRAM_CACHE[key]


def run(cfg, inputs, trace=False):
    in_maps, node_of_slot = preprocess(cfg, **inputs)
    nc = get_program(cfg)
    res = bass_utils.run_bass_kernel_spmd(
        nc, in_maps, core_ids=list(range(cfg.NC)), trace=trace)
    out = assemble_output(cfg, res.results, node_of_slot)
    return out, res


def kernel(**inputs) -> np.ndarray:
    out, _ = run(CFG_FULL, inputs)
    return out



# revision 28
# speedup vs baseline: 2.7416x; 1.3100x over previous
"""2-layer GCN (GCNConv+relu x2, linear head) on 8 Trainium2 NeuronCores.

Strategy (graph/data parallel, per sharding hint):
  - Nodes sharded across 8 cores by id; edges partitioned by destination.
  - Per core, destination nodes are bin-packed into B_FIX blocks of <=BLK
    dsts such that each (block, source-window) holds <= KCOL*128 edges.
    This gives an SPMD-uniform program; only tensor data varies per core.
  - Per layer: local matmul (x@W scaled by dinv) -> AllGather into a
    full node-major bf16 table in DRAM -> dma_gather one 256B element
    per edge = a PAIR of bf16 rows (slots 2w, 2w+1); parity-split
    selection matrices S_even/S_odd (is_equal on DVE, bf16) route the
    correct half; PE bf16 matmuls accumulate feature-major conv output
    in PSUM; self-loop terms enter via an identity-matmul transpose.
    Post: scale by dinv, +bias, relu, next-layer matmul (bf16).
  - The 4 source-window gathers go to 4 SWDGE queues: each queue's
    descriptor generation runs on its own GpSimd Q7 core pair, so the
    4 gathers of a batch overlap (queue 0 issued last since its pair
    is the one the engine timeline blocks on).
"""

import numpy as np

import concourse.bass as bass
import concourse.mybir as mybir
import concourse.tile as tile
from concourse import bacc
from concourse import bass_utils

import ml_dtypes

F32 = mybir.dt.float32
BF16 = mybir.dt.bfloat16
I16 = mybir.dt.int16
NP_BF16 = ml_dtypes.bfloat16


class Cfg:
    def __init__(self, n_nodes, in_feat, hidden, n_classes, n_cores, n_c,
                 blk, kcol, b_fix, nq, c_batch, self_dtype="bf16"):
        self.N = n_nodes
        self.IN_FEAT = in_feat
        self.HIDDEN = hidden
        self.N_CLASSES = n_classes
        self.NC = n_cores
        self.N_C = n_c                    # nodes per core (id // N_C)
        assert n_c * n_cores >= n_nodes
        self.BLK = blk                    # max dsts per block
        self.KCOL = kcol                  # columns per (block, stream)
        self.CAP = kcol * 128             # max edges per (block, stream)
        self.B_FIX = b_fix                # blocks per core (uniform)
        self.NQ = nq                      # source windows / gather streams
        self.SLOTS_C = b_fix * blk        # table slots per core
        assert self.SLOTS_C % 128 == 0
        self.NT = self.SLOTS_C // 128     # node tiles per core
        assert self.NT % 2 == 0
        self.TABLE_N = n_cores * self.SLOTS_C
        assert self.TABLE_N % nq == 0
        self.WIN = self.TABLE_N // nq     # table rows per source window
        assert self.WIN <= 32767          # int16 gather index range
        assert (n_cores % nq) == 0
        self.COLS_Q = b_fix * kcol        # gather columns per stream
        self.C_BATCH = c_batch            # columns per gather batch
        assert c_batch % kcol == 0 and self.COLS_Q % c_batch == 0
        self.N_BATCH = self.COLS_Q // c_batch
        self.BPB = c_batch // kcol        # blocks per batch
        assert self.BPB % 2 == 0          # block pairs never straddle batches
        self.SELF_DT = BF16 if self_dtype == "bf16" else F32
        self.NP_SELF = NP_BF16 if self_dtype == "bf16" else np.float32


CFG_FULL = Cfg(n_nodes=100000, in_feat=128, hidden=64, n_classes=16,
               n_cores=8, n_c=12544, blk=64, kcol=2, b_fix=224, nq=4,
               c_batch=28)


# ---------------------------------------------------------------------------
# Host-side preprocessing (sharding): all integer graph restructuring.
# ---------------------------------------------------------------------------

def preprocess(cfg, x, edge_index, W1, b1, W2, b2, Wl, bl):
    N, NC, N_C = cfg.N, cfg.NC, cfg.N_C
    src = np.asarray(edge_index[0]).astype(np.int64)
    dst = np.asarray(edge_index[1]).astype(np.int64)
    x = np.asarray(x, dtype=np.float32)

    deg = np.bincount(dst, minlength=N).astype(np.float32) + 1.0
    dinv = (1.0 / np.sqrt(deg)).astype(np.float32)

    # stream = stripe of the SOURCE node, chosen a priori and independent
    # of its parity class: stripe k nodes get packed into blocks
    # [56k, 56(k+1)) of their core, so table window q is the contiguous
    # rows [q*TABLE_N/NQ, ...) written by the q-th chunked AllGather.
    q_of = ((src // 2) % cfg.NQ).astype(np.int64)
    p_of = (src % 2).astype(np.int64)        # a-priori source parity class

    # per-(node, q, par) incoming edge counts
    degqp = np.bincount((dst * cfg.NQ + q_of) * 2 + p_of,
                        minlength=N * cfg.NQ * 2).reshape(N, cfg.NQ, 2)

    # --- per-core first-fit-decreasing packing of dsts into blocks ---
    # Constraints: per (block, q, par) <= 128 edges (one 128-row column);
    # node with id parity p gets a block position r with r % 2 == p, so
    # slot parity == id parity (known before packing any core).
    HBLK = cfg.BLK // 2
    B_STR = cfg.B_FIX // cfg.NQ          # blocks per stripe
    node_q = ((np.arange(NC * N_C) // 2) % cfg.NQ)
    slot_of = np.full(NC * N_C, -1, dtype=np.int64)
    node_of_slot = np.full(cfg.TABLE_N, -1, dtype=np.int64)
    for c in range(NC):
        lo, hi = c * N_C, min((c + 1) * N_C, N)
        if hi <= lo:
            continue
        for k in range(cfg.NQ):
            ids = lo + np.flatnonzero(node_q[lo:hi] == k)
            dq = degqp[ids].reshape(ids.size, cfg.NQ * 2)
            order = np.argsort(-dq.max(axis=1), kind="stable")
            accs = np.zeros((B_STR, cfg.NQ * 2), dtype=np.int64)
            cnts = np.zeros((B_STR, 2), dtype=np.int64)
            nopen = 1
            for j in order:
                v = dq[j]
                g = int(ids[j])
                pj = g % 2
                fits = (cnts[:nopen, pj] < HBLK) & \
                       np.all(accs[:nopen] + v <= 128, axis=1)
                w = np.flatnonzero(fits)
                if w.size == 0:
                    assert nopen < B_STR, \
                        f"core {c} stripe {k}: packing exceeds {B_STR} blocks"
                    b = nopen
                    nopen += 1
                else:
                    b = int(w[0])
                r = 2 * cnts[b, pj] + pj
                s = c * cfg.SLOTS_C + (k * B_STR + b) * cfg.BLK + r
                slot_of[g] = s
                node_of_slot[s] = g
                accs[b] += v
                cnts[b, pj] += 1

    slot_of = slot_of[:N]

    # --- per-core edge streams ---
    e_core = dst // N_C
    s_slot = slot_of[src]
    d_slot_l = slot_of[dst] - e_core * cfg.SLOTS_C
    e_b = d_slot_l // cfg.BLK
    e_r = d_slot_l % cfg.BLK

    P_Q = cfg.B_FIX * cfg.CAP            # positions per stream
    idx_all = np.zeros((NC, cfg.NQ, P_Q), dtype=np.int16)

    e_par = (s_slot % 2).astype(np.int64)    # == src % 2 by construction
    order2 = np.lexsort((e_par, e_b, q_of, e_core))
    es_c, eq_c, eb_c = e_core[order2], q_of[order2], e_b[order2]
    ep_c = e_par[order2]
    grp = ((es_c * cfg.NQ + eq_c) * cfg.B_FIX + eb_c) * 2 + ep_c
    _, start_idx, cnt_grp = np.unique(grp, return_index=True,
                                      return_counts=True)
    rank = np.arange(grp.size) - np.repeat(start_idx, cnt_grp)
    assert rank.max(initial=0) < 128
    # column = 2*block + parity; position = column*128 + rank
    pos = eb_c * cfg.CAP + ep_c * 128 + rank
    s_sorted = s_slot[order2]
    # table-window row of the source: window q holds, for every core c,
    # that core's stripe-q local rows at [c*3584, (c+1)*3584)
    STR_ROWS = cfg.SLOTS_C // cfg.NQ
    l_sorted = s_sorted % cfg.SLOTS_C
    assert np.all(l_sorted // STR_ROWS == eq_c)
    wrow = (s_sorted // cfg.SLOTS_C) * STR_ROWS + (l_sorted % STR_ROWS)
    # gather PAIR index (two table rows per 256B element)
    idx_val = (wrow // 2).astype(np.int16)
    idx_all[es_c, eq_c, pos] = idx_val

    # wrapped int16 layout: position i -> [i%16, i//16], replicated x8
    idx_w = idx_all.reshape(NC, cfg.NQ, -1, 16).transpose(0, 1, 3, 2)
    idx_dev = np.ascontiguousarray(np.tile(idx_w, (1, 1, 8, 1)))

    # host-built one-hot selection matrices, bf16:
    # S[core, q, 128, col*BLK + d] = 1 iff edge at (partition, col) has
    # dst-row d within its block. Padding positions stay all-zero.
    S_np = np.zeros((NC, cfg.NQ, cfg.COLS_Q, 128, cfg.BLK), dtype=NP_BF16)
    S_np[es_c, eq_c, pos // 128, pos % 128, e_r[order2]] = 1.0
    S_dev = np.ascontiguousarray(
        S_np.transpose(0, 1, 3, 2, 4).reshape(
            NC, cfg.NQ, 128, cfg.COLS_Q * cfg.BLK))
    del S_np

    # --- per-slot node data ---
    valid = node_of_slot >= 0
    xe = np.zeros((cfg.TABLE_N, cfg.IN_FEAT), dtype=np.float32)
    xe[valid] = x[node_of_slot[valid]]
    dinv_s = np.zeros(cfg.TABLE_N, dtype=np.float32)
    dinv_s[valid] = dinv[node_of_slot[valid]]

    W1 = np.asarray(W1, np.float32)
    W2 = np.asarray(W2, np.float32).astype(NP_BF16)
    Wl = np.asarray(Wl, np.float32).astype(NP_BF16)
    b1 = np.asarray(b1, np.float32)
    b2 = np.asarray(b2, np.float32)
    bl = np.asarray(bl, np.float32)

    ident2 = np.concatenate([np.eye(cfg.HIDDEN), np.eye(cfg.HIDDEN)],
                            axis=0).astype(cfg.NP_SELF)

    in_maps = []
    for c in range(NC):
        sl = slice(c * cfg.SLOTS_C, (c + 1) * cfg.SLOTS_C)
        dv = dinv_s[sl]
        m = {
            "xT": np.ascontiguousarray(xe[sl].T),
            "w1": W1, "w2": W2, "wl": Wl,
            "b1c": b1.reshape(-1, 1), "b2c": b2.reshape(-1, 1),
            "blrep": np.tile(bl[None, :], (128, 1)),
            "dinvn": np.ascontiguousarray(dv.reshape(cfg.NT, 128).T),
            "dinvfm": np.tile(dv[None, :], (cfg.HIDDEN, 1)),
            "ident2": ident2,
        }
        for q in range(cfg.NQ):
            m[f"idx{q}"] = idx_dev[c, q]
            m[f"S{q}"] = S_dev[c, q]
        in_maps.append(m)

    return in_maps, node_of_slot


def assemble_output(cfg, results, node_of_slot):
    out = np.zeros((cfg.N, cfg.N_CLASSES), dtype=np.float32)
    for c, r in enumerate(results):
        lg = r["logits"].reshape(128, cfg.NT, cfg.N_CLASSES)
        sl = node_of_slot[c * cfg.SLOTS_C:(c + 1) * cfg.SLOTS_C]\
            .reshape(cfg.NT, 128)
        for t in range(cfg.NT):
            v = sl[t] >= 0
            out[sl[t][v]] = lg[v, t, :]
    return out


# ---------------------------------------------------------------------------
# Device program
# ---------------------------------------------------------------------------

def build_program(cfg):
    nc = bacc.Bacc("TRN2", target_bir_lowering=False, debug=False,
                   num_devices=cfg.NC, num_swdge_queues=4)
    H, NT = cfg.HIDDEN, cfg.NT

    xT_d = nc.dram_tensor("xT", [cfg.IN_FEAT, cfg.SLOTS_C], F32,
                          kind="ExternalInput")
    w1_d = nc.dram_tensor("w1", [cfg.IN_FEAT, H], F32, kind="ExternalInput")
    w2_d = nc.dram_tensor("w2", [H, H], BF16, kind="ExternalInput")
    wl_d = nc.dram_tensor("wl", [H, cfg.N_CLASSES], BF16,
                          kind="ExternalInput")
    b1c_d = nc.dram_tensor("b1c", [H, 1], F32, kind="ExternalInput")
    b2c_d = nc.dram_tensor("b2c", [H, 1], F32, kind="ExternalInput")
    blrep_d = nc.dram_tensor("blrep", [128, cfg.N_CLASSES], F32,
                             kind="ExternalInput")
    dinvn_d = nc.dram_tensor("dinvn", [128, NT], F32, kind="ExternalInput")
    dinvfm_d = nc.dram_tensor("dinvfm", [H, cfg.SLOTS_C], F32,
                              kind="ExternalInput")
    ident_d = nc.dram_tensor("ident2", [128, H], cfg.SELF_DT,
                             kind="ExternalInput")
    idx_d = [nc.dram_tensor(f"idx{q}", [128, cfg.COLS_Q * 8], I16,
                            kind="ExternalInput") for q in range(cfg.NQ)]
    S_d = [nc.dram_tensor(f"S{q}", [128, cfg.COLS_Q * cfg.BLK], BF16,
                          kind="ExternalInput") for q in range(cfg.NQ)]
    logits_d = nc.dram_tensor("logits", [128, NT * cfg.N_CLASSES], F32,
                              kind="ExternalOutput")

    rg = [list(range(cfg.NC))]

    with tile.TileContext(nc) as tc:
        with tc.tile_pool(name="const", bufs=1) as cpool, \
             tc.tile_pool(name="dram", bufs=1, space="DRAM") as dpool, \
             tc.tile_pool(name="hp", bufs=3) as hpool:

            # hs chunk tiles: collective k fires as soon as its quarter of
            # the local shard is written, overlapping the producing layer.
            NCHUNK = 4
            CH = cfg.SLOTS_C // NCHUNK
            TPC = NT // NCHUNK            # 128-row tiles per chunk
            hs1_c = [dpool.tile([CH, H], BF16, tag=f"hs1c{k}",
                                name=f"hs1c{k}") for k in range(NCHUNK)]
            hs2_c = [dpool.tile([CH, H], BF16, tag=f"hs2c{k}",
                                name=f"hs2c{k}") for k in range(NCHUNK)]
            tab1_t = dpool.tile([cfg.TABLE_N, H], BF16, tag="tab1",
                                name="tab1_t")
            tab2_t = dpool.tile([cfg.TABLE_N, H], BF16, tag="tab2",
                                name="tab2_t")
            # chunk k of the table = contiguous rows (stripe-major layout)
            CHT = cfg.TABLE_N // NCHUNK
            tab1_v = [tab1_t[k * CHT:(k + 1) * CHT, :]
                      for k in range(NCHUNK)]
            tab2_v = [tab2_t[k * CHT:(k + 1) * CHT, :]
                      for k in range(NCHUNK)]

            def cload(dram, shape, dt, tag):
                t = cpool.tile(shape, dt, tag=tag)
                nc.sync.dma_start(out=t[:], in_=dram[:, :])
                return t

            w1_s = cload(w1_d, [cfg.IN_FEAT, H], F32, "w1")
            w2_s = cload(w2_d, [H, H], BF16, "w2")
            wl_s = cload(wl_d, [H, cfg.N_CLASSES], BF16, "wl")
            b1c_s = cload(b1c_d, [H, 1], F32, "b1c")
            b2c_s = cload(b2c_d, [H, 1], F32, "b2c")
            blrep_s = cload(blrep_d, [128, cfg.N_CLASSES], F32, "blrep")
            dinvn_s = cload(dinvn_d, [128, NT], F32, "dinvn")
            ident_s = cload(ident_d, [128, H], cfg.SELF_DT, "ident")

            self1_s = cpool.tile([128, NT * H], cfg.SELF_DT, tag="self1")
            self2_s = cpool.tile([128, NT * H], cfg.SELF_DT, tag="self2")
            stageL_s = cpool.tile([128, NT * cfg.N_CLASSES], F32, tag="stgL")

            # ---- phase A: table1 = dinv * (x @ W1), plus self terms ----
            with tc.tile_pool(name="xp", bufs=1) as xpool, \
                 tc.tile_pool(name="pA", bufs=2, space="PSUM") as pA:
                xt_s = xpool.tile([cfg.IN_FEAT, cfg.SLOTS_C], F32, tag="xt")
                nc.sync.dma_start(out=xt_s[:], in_=xT_d[:, :])
                for t in range(NT):
                    ps = pA.tile([128, H], F32, tag="a")
                    nc.tensor.matmul(out=ps[:],
                                     lhsT=xt_s[:, t * 128:(t + 1) * 128],
                                     rhs=w1_s[:], start=True, stop=True)
                    row = hpool.tile([128, H], BF16, tag="hsrow")
                    nc.vector.tensor_scalar_mul(out=row[:], in0=ps[:],
                                                scalar1=dinvn_s[:, t:t + 1])
                    k, tk = t // TPC, t % TPC
                    nc.sync.dma_start(
                        out=hs1_c[k][tk * 128:(tk + 1) * 128, :], in_=row[:])
                    nc.vector.tensor_scalar_mul(
                        out=self1_s[:, t * H:(t + 1) * H], in0=ps[:],
                        scalar1=dinvn_s[:, t:t + 1])
                    if tk == TPC - 1:
                        nc.gpsimd.collective_compute(
                            "AllGather", mybir.AluOpType.bypass,
                            replica_groups=rg, ins=[hs1_c[k][:, :]],
                            outs=[tab1_v[k]])

            # ---- phases B (layer1 -> table2) and C (layer2 -> logits) ----
            with tc.tile_pool(name="sp", bufs=3) as spool, \
                 tc.tile_pool(name="pp", bufs=2, space="PSUM") as pp:

                def conv_layer(layer):
                    tab_t = tab1_t if layer == 1 else tab2_t
                    self_s = self1_s if layer == 1 else self2_s
                    bc_s = b1c_s if layer == 1 else b2c_s
                    # paired-row view of the table: one 256B gather element
                    # covers two consecutive bf16 rows (slots 2w, 2w+1)
                    tabp = tab_t[:].rearrange("(n two) h -> n (two h)", two=2)
                    pair = {}
                    for i in range(cfg.N_BATCH):
                        msgs, Ss = [None] * cfg.NQ, [None] * cfg.NQ
                        for q in (list(range(1, cfg.NQ)) + [0]):
                            idx_t = spool.tile([128, cfg.C_BATCH * 8], I16,
                                               tag=f"idx{q}")
                            nc.sync.dma_start(
                                out=idx_t[:],
                                in_=idx_d[q][:, i * cfg.C_BATCH * 8:
                                             (i + 1) * cfg.C_BATCH * 8])
                            msg_t = spool.tile([128, cfg.C_BATCH, 2 * H],
                                               BF16, tag=f"msg{q}")
                            nc.gpsimd.dma_gather(
                                out_ap=msg_t[:],
                                in_ap=tabp[q * cfg.WIN // 2:
                                           (q + 1) * cfg.WIN // 2, :],
                                idxs_ap=idx_t[:],
                                num_idxs=cfg.C_BATCH * 128,
                                num_idxs_reg=cfg.C_BATCH * 128,
                                elem_size=2 * H, queue_num=q,
                                single_packet=False)
                            msgs[q] = msg_t[:].rearrange("p c f -> p (c f)")
                        for q in range(cfg.NQ):
                            S_t = spool.tile([128, cfg.C_BATCH * cfg.BLK],
                                             BF16, tag=f"S{q}")
                            nc.sync.dma_start(
                                out=S_t[:],
                                in_=S_d[q][:, i * cfg.C_BATCH * cfg.BLK:
                                           (i + 1) * cfg.C_BATCH * cfg.BLK])
                            Ss[q] = S_t[:]

                        dfm_t = spool.tile([H, cfg.BPB * cfg.BLK], F32,
                                           tag="dfm")
                        nc.sync.dma_start(
                            out=dfm_t[:],
                            in_=dinvfm_d[:, i * cfg.BPB * cfg.BLK:
                                         (i + 1) * cfg.BPB * cfg.BLK])

                        for bb in range(cfg.BPB):
                            b = i * cfg.BPB + bb
                            half = (b % 2) * H
                            t = b // 2
                            pfm = pp.tile([H, cfg.BLK], F32, tag="fm")
                            nc.tensor.matmul(
                                out=pfm[:],
                                lhsT=self_s[half:half + H,
                                            t * H:(t + 1) * H],
                                rhs=ident_s[half:half + H, :],
                                start=True, stop=False)
                            for q in range(cfg.NQ):
                                for k in range(cfg.KCOL):
                                    lc = bb * cfg.KCOL + k
                                    last = (q == cfg.NQ - 1 and
                                            k == cfg.KCOL - 1)
                                    # column parity k selects the half of
                                    # the gathered pair element
                                    nc.tensor.matmul(
                                        out=pfm[:],
                                        lhsT=msgs[q][:, lc * 2 * H + k * H:
                                                     lc * 2 * H +
                                                     (k + 1) * H],
                                        rhs=Ss[q][:, lc * cfg.BLK:
                                                  (lc + 1) * cfg.BLK],
                                        start=False, stop=last)
                            h_t = hpool.tile([H, cfg.BLK], F32, tag="h")
                            nc.vector.tensor_tensor(
                                out=h_t[:], in0=pfm[:],
                                in1=dfm_t[:, bb * cfg.BLK:(bb + 1) * cfg.BLK],
                                op=mybir.AluOpType.mult)
                            hr_t = hpool.tile([H, cfg.BLK], BF16, tag="hr")
                            nc.scalar.activation(
                                out=hr_t[:], in_=h_t[:],
                                func=mybir.ActivationFunctionType.Relu,
                                bias=bc_s[:])
                            if layer == 1:
                                if b % 2 == 0:
                                    pair["p2"] = pp.tile([128, H], F32, name="p2",
                                                         tag="pair")
                                p2 = pair["p2"]
                                nc.tensor.matmul(
                                    out=p2[half:half + H, :], lhsT=hr_t[:],
                                    rhs=w2_s[:], start=True, stop=True,
                                    tile_position=(0, half))
                                if b % 2 == 1:
                                    row2 = hpool.tile([128, H], BF16,
                                                      tag="hs2row")
                                    nc.vector.tensor_scalar_mul(
                                        out=row2[:], in0=p2[:],
                                        scalar1=dinvn_s[:, t:t + 1])
                                    k, tk = t // TPC, t % TPC
                                    nc.sync.dma_start(
                                        out=hs2_c[k][tk * 128:
                                                     (tk + 1) * 128, :],
                                        in_=row2[:])
                                    nc.vector.tensor_scalar_mul(
                                        out=self2_s[:, t * H:(t + 1) * H],
                                        in0=p2[:],
                                        scalar1=dinvn_s[:, t:t + 1])
                                    if tk == TPC - 1:
                                        nc.gpsimd.collective_compute(
                                            "AllGather",
                                            mybir.AluOpType.bypass,
                                            replica_groups=rg,
                                            ins=[hs2_c[k][:, :]],
                                            outs=[tab2_v[k]])
                            else:
                                if b % 2 == 0:
                                    pair["pl"] = pp.tile([128, cfg.N_CLASSES],
                                                         F32, name="pl", tag="pl")
                                pl = pair["pl"]
                                nc.tensor.matmul(
                                    out=pl[half:half + H, :], lhsT=hr_t[:],
                                    rhs=wl_s[:], start=True, stop=True,
                                    tile_position=(0, half))
                                if b % 2 == 1:
                                    nCL = cfg.N_CLASSES
                                    nc.vector.tensor_tensor(
                                        out=stageL_s[:, t * nCL:(t + 1) * nCL],
                                        in0=pl[:], in1=blrep_s[:],
                                        op=mybir.AluOpType.add)

                conv_layer(1)
                conv_layer(2)

            nc.sync.dma_start(out=logits_d[:, :], in_=stageL_s[:])

    nc.compile()
    return nc


_PROGRAM_CACHE = {}


def get_program(cfg):
    key = id(cfg)
    if key not in _PROGRAM_CACHE:
        _PROGRAM_CACHE[key] = build_program(cfg)
    return _PROGRAM_CACHE[key]


def run(cfg, inputs, trace=False):
    in_maps, node_of_slot = preprocess(cfg, **inputs)
    nc = get_program(cfg)
    res = bass_utils.run_bass_kernel_spmd(
        nc, in_maps, core_ids=list(range(cfg.NC)), trace=trace)
    out = assemble_output(cfg, res.results, node_of_slot)
    return out, res


def kernel(**inputs) -> np.ndarray:
    out, _ = run(CFG_FULL, inputs)
    return out



# revision 35
# speedup vs baseline: 2.7743x; 1.0119x over previous
"""2-layer GCN (GCNConv+relu x2, linear head) on 8 Trainium2 NeuronCores.

Strategy (graph/data parallel, per sharding hint):
  - Nodes sharded across 8 cores by id; edges partitioned by destination.
  - Per core, destination nodes are bin-packed into B_FIX blocks of <=BLK
    dsts such that each (block, source-window) holds <= KCOL*128 edges.
    This gives an SPMD-uniform program; only tensor data varies per core.
  - Per layer: local matmul (x@W scaled by dinv) -> AllGather into a
    full node-major bf16 table in DRAM -> dma_gather one 256B element
    per edge = a PAIR of bf16 rows (slots 2w, 2w+1); parity-split
    selection matrices S_even/S_odd (is_equal on DVE, bf16) route the
    correct half; PE bf16 matmuls accumulate feature-major conv output
    in PSUM; self-loop terms enter via an identity-matmul transpose.
    Post: scale by dinv, +bias, relu, next-layer matmul (bf16).
  - The 4 source-window gathers go to 4 SWDGE queues: each queue's
    descriptor generation runs on its own GpSimd Q7 core pair, so the
    4 gathers of a batch overlap (queue 0 issued last since its pair
    is the one the engine timeline blocks on).
"""

import numpy as np

import concourse.bass as bass
import concourse.mybir as mybir
import concourse.tile as tile
from concourse import bacc
from concourse import bass_utils

import ml_dtypes

F32 = mybir.dt.float32
BF16 = mybir.dt.bfloat16
I16 = mybir.dt.int16
NP_BF16 = ml_dtypes.bfloat16


class Cfg:
    def __init__(self, n_nodes, in_feat, hidden, n_classes, n_cores, n_c,
                 blk, kcol, b_fix, nq, c_batch, self_dtype="bf16"):
        self.N = n_nodes
        self.IN_FEAT = in_feat
        self.HIDDEN = hidden
        self.N_CLASSES = n_classes
        self.NC = n_cores
        self.N_C = n_c                    # nodes per core (id // N_C)
        assert n_c * n_cores >= n_nodes
        self.BLK = blk                    # max dsts per block
        self.KCOL = kcol                  # columns per (block, stream)
        self.CAP = kcol * 128             # max edges per (block, stream)
        self.B_FIX = b_fix                # blocks per core (uniform)
        self.NQ = nq                      # source windows / gather streams
        self.SLOTS_C = b_fix * blk        # table slots per core
        assert self.SLOTS_C % 128 == 0
        self.NT = self.SLOTS_C // 128     # node tiles per core
        assert self.NT % 2 == 0
        self.TABLE_N = n_cores * self.SLOTS_C
        assert self.TABLE_N % nq == 0
        self.WIN = self.TABLE_N // nq     # table rows per source window
        assert self.WIN <= 32767          # int16 gather index range
        assert (n_cores % nq) == 0
        self.COLS_Q = b_fix * kcol        # gather columns per stream
        self.C_BATCH = c_batch            # columns per gather batch
        assert c_batch % kcol == 0 and self.COLS_Q % c_batch == 0
        self.N_BATCH = self.COLS_Q // c_batch
        self.BPB = c_batch // kcol        # blocks per batch
        assert self.BPB % 2 == 0          # block pairs never straddle batches
        self.SELF_DT = BF16 if self_dtype == "bf16" else F32
        self.NP_SELF = NP_BF16 if self_dtype == "bf16" else np.float32


CFG_FULL = Cfg(n_nodes=100000, in_feat=128, hidden=64, n_classes=16,
               n_cores=8, n_c=12544, blk=64, kcol=2, b_fix=224, nq=4,
               c_batch=28)


# ---------------------------------------------------------------------------
# Host-side preprocessing (sharding): all integer graph restructuring.
# ---------------------------------------------------------------------------

def preprocess(cfg, x, edge_index, W1, b1, W2, b2, Wl, bl):
    N, NC, N_C = cfg.N, cfg.NC, cfg.N_C
    src = np.asarray(edge_index[0]).astype(np.int64)
    dst = np.asarray(edge_index[1]).astype(np.int64)
    x = np.asarray(x, dtype=np.float32)

    deg = np.bincount(dst, minlength=N).astype(np.float32) + 1.0
    dinv = (1.0 / np.sqrt(deg)).astype(np.float32)

    # stream = stripe of the SOURCE node, chosen a priori and independent
    # of its parity class: stripe k nodes get packed into blocks
    # [56k, 56(k+1)) of their core, so table window q is the contiguous
    # rows [q*TABLE_N/NQ, ...) written by the q-th chunked AllGather.
    q_of = ((src // 2) % cfg.NQ).astype(np.int64)
    p_of = (src % 2).astype(np.int64)        # a-priori source parity class

    # per-(node, q, par) incoming edge counts
    degqp = np.bincount((dst * cfg.NQ + q_of) * 2 + p_of,
                        minlength=N * cfg.NQ * 2).reshape(N, cfg.NQ, 2)

    # --- per-core first-fit-decreasing packing of dsts into blocks ---
    # Constraints: per (block, q, par) <= 128 edges (one 128-row column);
    # node with id parity p gets a block position r with r % 2 == p, so
    # slot parity == id parity (known before packing any core).
    HBLK = cfg.BLK // 2
    B_STR = cfg.B_FIX // cfg.NQ          # blocks per stripe
    node_q = ((np.arange(NC * N_C) // 2) % cfg.NQ)
    slot_of = np.full(NC * N_C, -1, dtype=np.int64)
    node_of_slot = np.full(cfg.TABLE_N, -1, dtype=np.int64)
    for c in range(NC):
        lo, hi = c * N_C, min((c + 1) * N_C, N)
        if hi <= lo:
            continue
        for k in range(cfg.NQ):
            ids = lo + np.flatnonzero(node_q[lo:hi] == k)
            dq = degqp[ids].reshape(ids.size, cfg.NQ * 2)
            order = np.argsort(-dq.max(axis=1), kind="stable")
            accs = np.zeros((B_STR, cfg.NQ * 2), dtype=np.int64)
            cnts = np.zeros((B_STR, 2), dtype=np.int64)
            nopen = 1
            for j in order:
                v = dq[j]
                g = int(ids[j])
                pj = g % 2
                fits = (cnts[:nopen, pj] < HBLK) & \
                       np.all(accs[:nopen] + v <= 128, axis=1)
                w = np.flatnonzero(fits)
                if w.size == 0:
                    assert nopen < B_STR, \
                        f"core {c} stripe {k}: packing exceeds {B_STR} blocks"
                    b = nopen
                    nopen += 1
                else:
                    b = int(w[0])
                r = 2 * cnts[b, pj] + pj
                s = c * cfg.SLOTS_C + (k * B_STR + b) * cfg.BLK + r
                slot_of[g] = s
                node_of_slot[s] = g
                accs[b] += v
                cnts[b, pj] += 1

    slot_of = slot_of[:N]

    # --- per-core edge streams ---
    e_core = dst // N_C
    s_slot = slot_of[src]
    d_slot_l = slot_of[dst] - e_core * cfg.SLOTS_C
    e_b = d_slot_l // cfg.BLK
    e_r = d_slot_l % cfg.BLK

    P_Q = cfg.B_FIX * cfg.CAP            # positions per stream
    idx_all = np.zeros((NC, cfg.NQ, P_Q), dtype=np.int16)

    e_par = (s_slot % 2).astype(np.int64)    # == src % 2 by construction
    order2 = np.lexsort((e_par, e_b, q_of, e_core))
    es_c, eq_c, eb_c = e_core[order2], q_of[order2], e_b[order2]
    ep_c = e_par[order2]
    grp = ((es_c * cfg.NQ + eq_c) * cfg.B_FIX + eb_c) * 2 + ep_c
    _, start_idx, cnt_grp = np.unique(grp, return_index=True,
                                      return_counts=True)
    rank = np.arange(grp.size) - np.repeat(start_idx, cnt_grp)
    assert rank.max(initial=0) < 128
    # column = 2*block + parity; position = column*128 + rank
    pos = eb_c * cfg.CAP + ep_c * 128 + rank
    s_sorted = s_slot[order2]
    # table-window row of the source: window q holds, for every core c,
    # that core's stripe-q local rows at [c*3584, (c+1)*3584)
    STR_ROWS = cfg.SLOTS_C // cfg.NQ
    l_sorted = s_sorted % cfg.SLOTS_C
    assert np.all(l_sorted // STR_ROWS == eq_c)
    wrow = (s_sorted // cfg.SLOTS_C) * STR_ROWS + (l_sorted % STR_ROWS)
    # gather PAIR index (two table rows per 256B element)
    idx_val = (wrow // 2).astype(np.int16)
    idx_all[es_c, eq_c, pos] = idx_val

    # wrapped int16 layout: position i -> [i%16, i//16], replicated x8
    idx_w = idx_all.reshape(NC, cfg.NQ, -1, 16).transpose(0, 1, 3, 2)
    idx_dev = np.ascontiguousarray(np.tile(idx_w, (1, 1, 8, 1)))

    # host-built one-hot selection matrices, bf16:
    # S[core, q, 128, col*BLK + d] = 1 iff edge at (partition, col) has
    # dst-row d within its block. Padding positions stay all-zero.
    S_np = np.zeros((NC, cfg.NQ, cfg.COLS_Q, 128, cfg.BLK), dtype=NP_BF16)
    S_np[es_c, eq_c, pos // 128, pos % 128, e_r[order2]] = 1.0
    S_dev = np.ascontiguousarray(
        S_np.transpose(0, 1, 3, 2, 4).reshape(
            NC, cfg.NQ, 128, cfg.COLS_Q * cfg.BLK))
    del S_np

    # --- per-slot node data ---
    valid = node_of_slot >= 0
    xe = np.zeros((cfg.TABLE_N, cfg.IN_FEAT), dtype=np.float32)
    xe[valid] = x[node_of_slot[valid]]
    dinv_s = np.zeros(cfg.TABLE_N, dtype=np.float32)
    dinv_s[valid] = dinv[node_of_slot[valid]]

    W1 = np.asarray(W1, np.float32)
    W2 = np.asarray(W2, np.float32).astype(NP_BF16)
    Wl = np.asarray(Wl, np.float32).astype(NP_BF16)
    b1 = np.asarray(b1, np.float32)
    b2 = np.asarray(b2, np.float32)
    bl = np.asarray(bl, np.float32)

    ident2 = np.concatenate([np.eye(cfg.HIDDEN), np.eye(cfg.HIDDEN)],
                            axis=0).astype(cfg.NP_SELF)

    in_maps = []
    for c in range(NC):
        sl = slice(c * cfg.SLOTS_C, (c + 1) * cfg.SLOTS_C)
        dv = dinv_s[sl]
        m = {
            "xT": np.ascontiguousarray(xe[sl].T),
            "w1": W1, "w2": W2, "wl": Wl,
            "b1c": b1.reshape(-1, 1), "b2c": b2.reshape(-1, 1),
            "blrep": np.tile(bl[None, :], (128, 1)),
            "dinvn": np.ascontiguousarray(dv.reshape(cfg.NT, 128).T),
            "dinvfm": np.tile(dv[None, :], (cfg.HIDDEN, 1)),
            "ident2": ident2,
        }
        for q in range(cfg.NQ):
            m[f"idx{q}"] = idx_dev[c, q]
            m[f"S{q}"] = S_dev[c, q]
        in_maps.append(m)

    return in_maps, node_of_slot


def assemble_output(cfg, results, node_of_slot):
    out = np.zeros((cfg.N, cfg.N_CLASSES), dtype=np.float32)
    for c, r in enumerate(results):
        lg = r["logits"].reshape(128, cfg.NT, cfg.N_CLASSES)
        sl = node_of_slot[c * cfg.SLOTS_C:(c + 1) * cfg.SLOTS_C]\
            .reshape(cfg.NT, 128)
        for t in range(cfg.NT):
            v = sl[t] >= 0
            out[sl[t][v]] = lg[v, t, :]
    return out


# ---------------------------------------------------------------------------
# Device program
# ---------------------------------------------------------------------------

def build_program(cfg):
    nc = bacc.Bacc("TRN2", target_bir_lowering=False, debug=False,
                   num_devices=cfg.NC, num_swdge_queues=4)
    H, NT = cfg.HIDDEN, cfg.NT

    xT_d = nc.dram_tensor("xT", [cfg.IN_FEAT, cfg.SLOTS_C], F32,
                          kind="ExternalInput")
    w1_d = nc.dram_tensor("w1", [cfg.IN_FEAT, H], F32, kind="ExternalInput")
    w2_d = nc.dram_tensor("w2", [H, H], BF16, kind="ExternalInput")
    wl_d = nc.dram_tensor("wl", [H, cfg.N_CLASSES], BF16,
                          kind="ExternalInput")
    b1c_d = nc.dram_tensor("b1c", [H, 1], F32, kind="ExternalInput")
    b2c_d = nc.dram_tensor("b2c", [H, 1], F32, kind="ExternalInput")
    blrep_d = nc.dram_tensor("blrep", [128, cfg.N_CLASSES], F32,
                             kind="ExternalInput")
    dinvn_d = nc.dram_tensor("dinvn", [128, NT], F32, kind="ExternalInput")
    dinvfm_d = nc.dram_tensor("dinvfm", [H, cfg.SLOTS_C], F32,
                              kind="ExternalInput")
    ident_d = nc.dram_tensor("ident2", [128, H], cfg.SELF_DT,
                             kind="ExternalInput")
    idx_d = [nc.dram_tensor(f"idx{q}", [128, cfg.COLS_Q * 8], I16,
                            kind="ExternalInput") for q in range(cfg.NQ)]
    S_d = [nc.dram_tensor(f"S{q}", [128, cfg.COLS_Q * cfg.BLK], BF16,
                          kind="ExternalInput") for q in range(cfg.NQ)]
    logits_d = nc.dram_tensor("logits", [128, NT * cfg.N_CLASSES], F32,
                              kind="ExternalOutput")

    rg = [list(range(cfg.NC))]

    with tile.TileContext(nc) as tc:
        with tc.tile_pool(name="const", bufs=1) as cpool, \
             tc.tile_pool(name="dram", bufs=1, space="DRAM") as dpool, \
             tc.tile_pool(name="hp", bufs=3) as hpool:

            # hs chunk tiles: collective k fires as soon as its quarter of
            # the local shard is written, overlapping the producing layer.
            NCHUNK = 4
            CH = cfg.SLOTS_C // NCHUNK
            TPC = NT // NCHUNK            # 128-row tiles per chunk
            hs1_c = [dpool.tile([CH, H], BF16, tag=f"hs1c{k}",
                                name=f"hs1c{k}") for k in range(NCHUNK)]
            hs2_c = [dpool.tile([CH, H], BF16, tag=f"hs2c{k}",
                                name=f"hs2c{k}") for k in range(NCHUNK)]
            tab1_t = dpool.tile([cfg.TABLE_N, H], BF16, tag="tab1",
                                name="tab1_t")
            tab2_t = dpool.tile([cfg.TABLE_N, H], BF16, tag="tab2",
                                name="tab2_t")
            # chunk k of the table = contiguous rows (stripe-major layout)
            CHT = cfg.TABLE_N // NCHUNK
            tab1_v = [tab1_t[k * CHT:(k + 1) * CHT, :]
                      for k in range(NCHUNK)]
            tab2_v = [tab2_t[k * CHT:(k + 1) * CHT, :]
                      for k in range(NCHUNK)]

            def cload(dram, shape, dt, tag):
                t = cpool.tile(shape, dt, tag=tag)
                nc.sync.dma_start(out=t[:], in_=dram[:, :])
                return t

            w1_s = cload(w1_d, [cfg.IN_FEAT, H], F32, "w1")
            w2_s = cload(w2_d, [H, H], BF16, "w2")
            wl_s = cload(wl_d, [H, cfg.N_CLASSES], BF16, "wl")
            b1c_s = cload(b1c_d, [H, 1], F32, "b1c")
            b2c_s = cload(b2c_d, [H, 1], F32, "b2c")
            blrep_s = cload(blrep_d, [128, cfg.N_CLASSES], F32, "blrep")
            dinvn_s = cload(dinvn_d, [128, NT], F32, "dinvn")
            ident_s = cload(ident_d, [128, H], cfg.SELF_DT, "ident")

            self1_s = cpool.tile([128, NT * H], cfg.SELF_DT, tag="self1")
            self2_s = cpool.tile([128, NT * H], cfg.SELF_DT, tag="self2")
            stageL_s = cpool.tile([128, NT * cfg.N_CLASSES], F32, tag="stgL")

            # ---- phase A: table1 = dinv * (x @ W1), plus self terms ----
            with tc.tile_pool(name="xp", bufs=1) as xpool, \
                 tc.tile_pool(name="pA", bufs=2, space="PSUM") as pA:
                xt_s = xpool.tile([cfg.IN_FEAT, cfg.SLOTS_C], F32, tag="xt")
                for k in range(NCHUNK):
                    nc.sync.dma_start(out=xt_s[:, k * CH:(k + 1) * CH],
                                      in_=xT_d[:, k * CH:(k + 1) * CH])
                for t in range(NT):
                    ps = pA.tile([128, H], F32, tag="a")
                    nc.tensor.matmul(out=ps[:],
                                     lhsT=xt_s[:, t * 128:(t + 1) * 128],
                                     rhs=w1_s[:], start=True, stop=True)
                    row = hpool.tile([128, H], BF16, tag="hsrow")
                    nc.vector.tensor_scalar_mul(out=row[:], in0=ps[:],
                                                scalar1=dinvn_s[:, t:t + 1])
                    k, tk = t // TPC, t % TPC
                    nc.sync.dma_start(
                        out=hs1_c[k][tk * 128:(tk + 1) * 128, :], in_=row[:])
                    nc.vector.tensor_scalar_mul(
                        out=self1_s[:, t * H:(t + 1) * H], in0=ps[:],
                        scalar1=dinvn_s[:, t:t + 1])
                    if tk == TPC - 1:
                        nc.gpsimd.collective_compute(
                            "AllGather", mybir.AluOpType.bypass,
                            replica_groups=rg, ins=[hs1_c[k][:, :]],
                            outs=[tab1_v[k]])

            # ---- phases B (layer1 -> table2) and C (layer2 -> logits) ----
            with tc.tile_pool(name="sp", bufs=3) as spool, \
                 tc.tile_pool(name="pp", bufs=2, space="PSUM") as pp:

                def conv_layer(layer):
                    tab_t = tab1_t if layer == 1 else tab2_t
                    self_s = self1_s if layer == 1 else self2_s
                    bc_s = b1c_s if layer == 1 else b2c_s
                    # paired-row view of the table: one 256B gather element
                    # covers two consecutive bf16 rows (slots 2w, 2w+1)
                    tabp = tab_t[:].rearrange("(n two) h -> n (two h)", two=2)
                    pair = {}
                    fired = [False] * NCHUNK

                    def fire_tab2(k):
                        nc.gpsimd.collective_compute(
                            "AllGather", mybir.AluOpType.bypass,
                            replica_groups=rg, ins=[hs2_c[k][:, :]],
                            outs=[tab2_v[k]])
                        fired[k] = True

                    for i in range(cfg.N_BATCH):
                        msgs, Ss = [None] * cfg.NQ, [None] * cfg.NQ
                        for q in (list(range(1, cfg.NQ)) + [0]):
                            idx_t = spool.tile([128, cfg.C_BATCH * 8], I16,
                                               tag=f"idx{q}")
                            nc.sync.dma_start(
                                out=idx_t[:],
                                in_=idx_d[q][:, i * cfg.C_BATCH * 8:
                                             (i + 1) * cfg.C_BATCH * 8])
                            msg_t = spool.tile([128, cfg.C_BATCH, 2 * H],
                                               BF16, tag=f"msg{q}")
                            nc.gpsimd.dma_gather(
                                out_ap=msg_t[:],
                                in_ap=tabp[q * cfg.WIN // 2:
                                           (q + 1) * cfg.WIN // 2, :],
                                idxs_ap=idx_t[:],
                                num_idxs=cfg.C_BATCH * 128,
                                num_idxs_reg=cfg.C_BATCH * 128,
                                elem_size=2 * H, queue_num=q,
                                single_packet=False)
                            msgs[q] = msg_t[:].rearrange("p c f -> p (c f)")
                        if layer == 1:
                            # fire chunk collectives two batches after their
                            # last row landed, AFTER this batch's gathers, so
                            # the (in-order) Pool trigger never blocks them
                            for k in range(NCHUNK):
                                if not fired[k] and i >= 4 * k + 5:
                                    fire_tab2(k)
                        for q in range(cfg.NQ):
                            S_t = spool.tile([128, cfg.C_BATCH * cfg.BLK],
                                             BF16, tag=f"S{q}")
                            nc.sync.dma_start(
                                out=S_t[:],
                                in_=S_d[q][:, i * cfg.C_BATCH * cfg.BLK:
                                           (i + 1) * cfg.C_BATCH * cfg.BLK])
                            Ss[q] = S_t[:]

                        dfm_t = spool.tile([H, cfg.BPB * cfg.BLK], F32,
                                           tag="dfm")
                        nc.sync.dma_start(
                            out=dfm_t[:],
                            in_=dinvfm_d[:, i * cfg.BPB * cfg.BLK:
                                         (i + 1) * cfg.BPB * cfg.BLK])

                        for bb in range(cfg.BPB):
                            b = i * cfg.BPB + bb
                            half = (b % 2) * H
                            t = b // 2
                            pfm = pp.tile([H, cfg.BLK], F32, tag="fm")
                            nc.tensor.matmul(
                                out=pfm[:],
                                lhsT=self_s[half:half + H,
                                            t * H:(t + 1) * H],
                                rhs=ident_s[half:half + H, :],
                                start=True, stop=False)
                            for q in range(cfg.NQ):
                                for k in range(cfg.KCOL):
                                    lc = bb * cfg.KCOL + k
                                    last = (q == cfg.NQ - 1 and
                                            k == cfg.KCOL - 1)
                                    # column parity k selects the half of
                                    # the gathered pair element
                                    nc.tensor.matmul(
                                        out=pfm[:],
                                        lhsT=msgs[q][:, lc * 2 * H + k * H:
                                                     lc * 2 * H +
                                                     (k + 1) * H],
                                        rhs=Ss[q][:, lc * cfg.BLK:
                                                  (lc + 1) * cfg.BLK],
                                        start=False, stop=last)
                            h_t = hpool.tile([H, cfg.BLK], F32, tag="h")
                            nc.vector.tensor_tensor(
                                out=h_t[:], in0=pfm[:],
                                in1=dfm_t[:, bb * cfg.BLK:(bb + 1) * cfg.BLK],
                                op=mybir.AluOpType.mult)
                            hr_t = hpool.tile([H, cfg.BLK], BF16, tag="hr")
                            nc.scalar.activation(
                                out=hr_t[:], in_=h_t[:],
                                func=mybir.ActivationFunctionType.Relu,
                                bias=bc_s[:])
                            if layer == 1:
                                if b % 2 == 0:
                                    pair["p2"] = pp.tile([128, H], F32, name="p2",
                                                         tag="pair")
                                p2 = pair["p2"]
                                nc.tensor.matmul(
                                    out=p2[half:half + H, :], lhsT=hr_t[:],
                                    rhs=w2_s[:], start=True, stop=True,
                                    tile_position=(0, half))
                                if b % 2 == 1:
                                    row2 = hpool.tile([128, H], BF16,
                                                      tag="hs2row")
                                    nc.vector.tensor_scalar_mul(
                                        out=row2[:], in0=p2[:],
                                        scalar1=dinvn_s[:, t:t + 1])
                                    k, tk = t // TPC, t % TPC
                                    nc.sync.dma_start(
                                        out=hs2_c[k][tk * 128:
                                                     (tk + 1) * 128, :],
                                        in_=row2[:])
                                    nc.vector.tensor_scalar_mul(
                                        out=self2_s[:, t * H:(t + 1) * H],
                                        in0=p2[:],
                                        scalar1=dinvn_s[:, t:t + 1])
                            else:
                                if b % 2 == 0:
                                    pair["pl"] = pp.tile([128, cfg.N_CLASSES],
                                                         F32, name="pl", tag="pl")
                                pl = pair["pl"]
                                nc.tensor.matmul(
                                    out=pl[half:half + H, :], lhsT=hr_t[:],
                                    rhs=wl_s[:], start=True, stop=True,
                                    tile_position=(0, half))
                                if b % 2 == 1:
                                    nCL = cfg.N_CLASSES
                                    nc.vector.tensor_tensor(
                                        out=stageL_s[:, t * nCL:(t + 1) * nCL],
                                        in0=pl[:], in1=blrep_s[:],
                                        op=mybir.AluOpType.add)

                    if layer == 1:
                        for k in range(NCHUNK):
                            if not fired[k]:
                                fire_tab2(k)

                conv_layer(1)
                conv_layer(2)

            nc.sync.dma_start(out=logits_d[:, :], in_=stageL_s[:])

    nc.compile()
    return nc


_PROGRAM_CACHE = {}


def get_program(cfg):
    key = id(cfg)
    if key not in _PROGRAM_CACHE:
        _PROGRAM_CACHE[key] = build_program(cfg)
    return _PROGRAM_CACHE[key]


def run(cfg, inputs, trace=False):
    in_maps, node_of_slot = preprocess(cfg, **inputs)
    nc = get_program(cfg)
    res = bass_utils.run_bass_kernel_spmd(
        nc, in_maps, core_ids=list(range(cfg.NC)), trace=trace)
    out = assemble_output(cfg, res.results, node_of_slot)
    return out, res


def kernel(**inputs) -> np.ndarray:
    out, _ = run(CFG_FULL, inputs)
    return out

